# revision 18
# baseline (speedup 1.0000x reference)
"""BiAttentionMRU Trainium2 kernel.

Data-parallel over batch: B=16 -> 2 batch elements on each of 8 cores.
All weights replicated; the embedding is host-compacted to each core's
working set (~4k unique tokens) so the on-device gather can use the
batched SWDGE dma_gather in TRANSPOSE mode, which lands the article
directly in [d-on-partitions, token-cols] layout (no PE transposes) and
costs ~1us of gpsimd issue per 2048 tokens instead of ~17us.

Layouts: everything is [d, t] with d padded 300->384 = 3 chunks of 128
(pad rows are zeros end-to-end: emb pad cols, weight pad rows/cols and
bias pad rows are all zero, so pad lanes carry exact zeros through
z/o/gate/scan/attention).

Pipeline: a burst of zero-weight dummy matmuls at t=0 trips the PE HAM
clock gate to 2.4GHz before the real GEMMs arrive; z/o/B1 + CE stream
per batch as the gathers land; the 5->3->1 gate mix runs as
scaled-identity accumulating matmuls in 500-col chunks whose relus are
round-robined over Scalar/Vector/GpSimd; the MRU scan runs in 500-col
carry-chained chunks that chase the gate chunks, and the attention
(same exp/Z-folding algebra as before, 500-col chunks) chases the scan,
interleaved across the two batch elements to keep PE dense.
"""

import sys

sys.path.insert(0, "/opt/trn_rl_repo")

import numpy as np
import ml_dtypes

import concourse.bass as bass
import concourse.tile as tile
from concourse import bacc, mybir
from concourse.masks import make_identity

F32 = mybir.dt.float32
BF16 = mybir.dt.bfloat16
I16 = mybir.dt.int16
AX = mybir.AxisListType
OP = mybir.AluOpType
AF = mybir.ActivationFunctionType

DIM = 300
DPAD = 384
B_FULL = 16
NCORES = 8
BPC = B_FULL // NCORES  # batch per core = 2
T = 2000
TA = 2048               # article gather length (48 pad idx-0 tokens)
TQ = 30
TO = 16
RANGES = (1, 2, 4, 10, 25)
RMAX = 4608             # compacted per-core vocab rows (>= ~4.1k uniques)

DCS = 128
DC = 3

DT = BF16
NPDT = ml_dtypes.bfloat16

# z/o/B1 psum pairs (matmuls at <=512 cols, one ACT per 2-bank pair)
TP = [(0, 1024, (512, 512)), (1024, 976, (512, 464))]
# mix / scan / attention 500-col chunks
NSC = 4
SCW = 500
TSUB = 125  # attention sub-chunk (partitions of the s2 block)

# ---- packed bf16 weights: [128, 11520] ----
W_ART = 0            # 3 kc * (z|o|ce0) * 384
W_CE = 3456          # 3 kc * 4 ri * 384
W_F1 = 8064          # 3 kc * 384 (transposed)
W_F2 = 9216          # untransposed
W_F3 = 10368
WCOLS = 11520

# ---- packed f32 tensor: [128, 506] ----
F_BIAS = 0           # dc*10 + {0 bz, 1 bo, 2..6 ce_b[0..4]}
F_SCAL = 30          # 24 scalar cols (below)
F_AS1 = 54           # 6 blocks of 75 (block j = fi*3+dc)
F_AS2 = 504          # rows 0..74
F_BAS1 = 505         # rows 0..74
FCOLS = 506

SC_M1 = 0            # 15 cols: m1[k,r]/r at 5k+ri
SC_M1B = 15          # 3 cols: mr1_b
SC_M2 = 18           # 3 cols: mr2_W
SC_M2B = 21          # mr2_b
SC_AS2B = 22         # as2_b

# ---- packed i16 indices: [128, 280], idx i of a list at [i%16, base+i//16]
IX_QOPT = 0          # 256: q_b0(30+2), q_b1(30+2), opt_b0(64), opt_b1(64), pad
IX_OG0 = 16          # 64: b0 options, row layout
IX_OG1 = 20          # 64: b1 options
IX_ART0 = 24         # 2048: b0 article (+48 pad)
IX_ART1 = 152        # 2048: b1 article
IXCOLS = 280

N_WARM = 28          # dummy matmuls to trip the HAM clock gate at t=0


def _build_program():
    nc = bacc.Bacc("TRN2", target_bir_lowering=False, debug=False,
                   num_devices=NCORES)

    embc = nc.dram_tensor("embc", [RMAX, DPAD], DT, kind="ExternalInput")
    idx_pack = nc.dram_tensor("idx_pack", [128, IXCOLS], I16,
                              kind="ExternalInput")
    wpack = nc.dram_tensor("wpack", [128, WCOLS], DT, kind="ExternalInput")
    fpack = nc.dram_tensor("fpack", [128, FCOLS], F32, kind="ExternalInput")
    out = nc.dram_tensor("scores", [BPC, 4], F32, kind="ExternalOutput")

    with tile.TileContext(nc) as tc:
        from contextlib import ExitStack
        with ExitStack() as ctx:
            _emit(nc, tc, ctx, embc, idx_pack, wpack, fpack, out)

    nc.compile()
    return nc


def _emit(nc, tc, ctx, embc, idx_pack, wpack, fpack, out):
    # ---------------- pools ----------------
    consts = ctx.enter_context(tc.tile_pool(name="consts", bufs=1))
    persist = ctx.enter_context(tc.tile_pool(name="persist", bufs=1))
    p_art = ctx.enter_context(tc.tile_pool(name="p_art", bufs=8))
    p_zob = ctx.enter_context(tc.tile_pool(name="p_zob", bufs=2))
    p_xs = ctx.enter_context(tc.tile_pool(name="p_xs", bufs=2))
    p_h1 = ctx.enter_context(tc.tile_pool(name="p_h1", bufs=2))
    p_gate = ctx.enter_context(tc.tile_pool(name="p_gate", bufs=3))
    p_gz = ctx.enter_context(tc.tile_pool(name="p_gz", bufs=2))
    small = ctx.enter_context(tc.tile_pool(name="small", bufs=2))
    # PSUM (8 banks): pp2 2x2-bank (zob/CE-r2/pb) + mix 2x1 + work 2x1
    pp2 = ctx.enter_context(tc.tile_pool(name="pp2", bufs=2, space="PSUM"))
    pp_m = ctx.enter_context(tc.tile_pool(name="pp_m", bufs=2, space="PSUM"))
    pp_w = ctx.enter_context(tc.tile_pool(name="pp_w", bufs=2, space="PSUM"))

    # ---------------- HAM warm-up: dummy matmuls on zeroed tiles ----------
    wz_l = consts.tile([128, 128], DT)
    wz_r = consts.tile([128, 512], DT)
    nc.vector.memset(wz_l[:], 0.0)
    nc.vector.memset(wz_r[:], 0.0)
    for _ in range(N_WARM):
        ps = pp_w.tile([128, 512], F32, tag="w")
        nc.tensor.matmul(ps[:], wz_l[:], wz_r[:], start=True, stop=True)

    # ---------------- index pack + gathers ----------------
    ixp = consts.tile([128, IXCOLS], I16)
    nc.sync.dma_start(ixp[:], idx_pack[:])

    def gather(out_ap, ixcol, n, transpose):
        nc.gpsimd.dma_gather(
            out_ap, embc[:], ixp[:, ixcol:ixcol + (n + 15) // 16],
            n, n, DPAD, transpose=transpose)

    qoptT = persist.tile([128, DC, 256], DT, tag="qoptT")
    gather(qoptT[:], IX_QOPT, 256, True)
    og = [persist.tile([128, 1, DPAD], DT, tag=f"og{b}", name=f"og{b}")
          for b in range(BPC)]
    gather(og[0][:], IX_OG0, 64, False)
    gather(og[1][:], IX_OG1, 64, False)
    # article in 512-token chunks: one transpose gather pushes one tx
    # descriptor per index and the SWDGE ring wedges above ~512
    artT = [[p_art.tile([128, DC, 512], DT, tag="artT", name=f"artT{b}_{c}")
             for c in range(4)] for b in range(BPC)]
    for b, base in ((0, IX_ART0), (1, IX_ART1)):
        for c in range(4):
            gather(artT[b][c][:], base + 32 * c, 512, True)

    # q/opt transposed views (cols within qoptT)
    def qT(b):           # [128, DC, 30]
        return qoptT[:, :, 32 * b:32 * b + TQ]

    def oT(b):           # [128, DC, 64] = (o w)
        return qoptT[:, :, 64 + 64 * b:128 + 64 * b]

    # ---------------- weights ----------------
    wp = consts.tile([128, WCOLS], DT)
    nc.sync.dma_start(wp[:], wpack[:])
    fp = consts.tile([128, FCOLS], F32)
    nc.sync.dma_start(fp[:], fpack[:])

    def w_art_v(kc, s, dc):
        o = W_ART + kc * 1152 + s * DPAD + dc * DCS
        return wp[:, o:o + DCS]

    def w_ce_v(kc, ri, dc):
        o = W_CE + kc * 1536 + ri * DPAD + dc * DCS
        return wp[:, o:o + DCS]

    def w_f_v(base, kc):
        return wp[:, base + kc * DPAD:base + (kc + 1) * DPAD]

    def bias(dc, col):
        return fp[:, dc * 10 + col:dc * 10 + col + 1]

    def sc(col):
        return fp[:, F_SCAL + col:F_SCAL + col + 1]

    ident = consts.tile([128, 128], DT)
    make_identity(nc, ident[:])

    # scaled identities for the PE-side gate mix
    mI = consts.tile([128, 18, 128], DT)
    for j in range(18):
        scol = (SC_M1 + j) if j < 15 else (SC_M2 + j - 15)
        nc.vector.tensor_scalar_mul(mI[:, j, :], ident[:], sc(scol))

    # ---------------- attention prep (needs only qoptT) ----------------
    k1T = [persist.tile([128, DC, TQ], DT, tag=f"k1T{b}", name=f"k1T{b}")
           for b in range(BPC)]
    qk_sb = [persist.tile([TQ, 132], DT, tag=f"qk{b}", name=f"qk{b}")
             for b in range(BPC)]
    for b in range(BPC):
        for dc in range(DC):
            ps = pp_w.tile([128, 512], F32, tag="w")
            for kc in range(DC):
                nc.tensor.matmul(ps[:, :TQ],
                                 w_f_v(W_F1, kc)[:, dc * DCS:(dc + 1) * DCS],
                                 qT(b)[:, kc, :], start=(kc == 0),
                                 stop=(kc == DC - 1))
            nc.scalar.copy(k1T[b][:, dc, :], ps[:, :TQ])

        aTs = []
        for fi, base in enumerate((W_F2, W_F3)):
            a_ps = pp_w.tile([TQ, DPAD], F32, tag="w")
            for kc in range(DC):
                nc.tensor.matmul(a_ps[:], qT(b)[:, kc, :], w_f_v(base, kc),
                                 start=(kc == 0), stop=(kc == DC - 1))
            a_sb = small.tile([TQ, DPAD], DT, tag="a_sb")
            nc.vector.tensor_copy(a_sb[:], a_ps[:])
            aT = persist.tile([128, DC, TQ], DT, tag=f"aT{fi}_{b}", name=f"aT{fi}_{b}")
            for dc in range(DC):
                tp = pp_w.tile([128, 512], DT, tag="w")
                nc.tensor.transpose(tp[:, :TQ],
                                    a_sb[:, dc * DCS:(dc + 1) * DCS],
                                    ident[:TQ, :TQ])
                nc.vector.tensor_copy(aT[:, dc, :], tp[:, :TQ])
            aTs.append(aT)

        qk_ps = pp_w.tile([TQ, 512], F32, tag="w")
        for fi in range(2):
            for kc in range(DC):
                nc.tensor.matmul(qk_ps[:, 64 * fi:64 * fi + 64],
                                 aTs[fi][:, kc, :], oT(b)[:, kc, :],
                                 start=(kc == 0), stop=(kc == DC - 1))
        nc.vector.tensor_copy(qk_sb[b][:, 0:128], qk_ps[:, :128])
        nc.vector.memset(qk_sb[b][:, 128:132], 1.0)

    # ---------------- group sums (xs_r in [d, g]) ----------------
    # xs2 per 512-tile (pairs are 2-aligned); xs4/xs10 from xs2; xs25 full
    # groups per tile + 3 straddle groups patched from xs2 plus one article
    # column (25g odd/even cases worked out per straddle).
    TW = [512, 512, 512, 464]
    xs = [None] * BPC
    for b in range(BPC):
        a = artT[b]
        xs2 = p_xs.tile([128, DC, T // 2], DT, tag="xs2", name=f"xs2_{b}")
        xs4 = p_xs.tile([128, DC, T // 4], DT, tag="xs4", name=f"xs4_{b}")
        xs10 = p_xs.tile([128, DC, T // 10], DT, tag="xs10", name=f"xs10_{b}")
        xs25 = p_xs.tile([128, DC, T // 25], DT, tag="xs25", name=f"xs25_{b}")
        with nc.allow_low_precision(reason="bf16 group sums"):
            for dc in range(DC):
                for c in range(4):
                    w = TW[c]
                    nc.vector.tensor_add(
                        xs2[:, dc, 256 * c:256 * c + w // 2],
                        a[c][:, dc, 0:w:2], a[c][:, dc, 1:w:2])
                    t0 = 512 * c
                    gs, ge = -(-t0 // 25), (t0 + w) // 25
                    nc.vector.tensor_reduce(
                        xs25[:, dc, gs:ge],
                        a[c][:, dc, 25 * gs - t0:25 * ge - t0].rearrange(
                            "p (g r) -> p g r", r=25),
                        AX.X, OP.add)
                for r0 in range(0, T, 500):
                    h0, h1r = r0 // 2, (r0 + 500) // 2
                    nc.gpsimd.tensor_add(xs4[:, dc, r0 // 4:(r0 + 500) // 4],
                                         xs2[:, dc, h0:h1r:2],
                                         xs2[:, dc, h0 + 1:h1r:2])
                    nc.vector.tensor_reduce(
                        xs10[:, dc, r0 // 10:(r0 + 500) // 10],
                        xs2[:, dc, h0:h1r].rearrange("p (g r) -> p g r", r=5),
                        AX.X, OP.add)
                # straddle groups: (g, xs2 col range, art tile, art col)
                for g, x0, ac, acol in ((20, 250, 1, 12), (40, 500, 2, 0),
                                        (61, 763, 2, 501)):
                    tmp = small.tile([128, 1], DT, tag="s25", name="s25")
                    nc.vector.tensor_reduce(
                        tmp[:, :],
                        xs2[:, dc, x0:x0 + 12].rearrange(
                            "p (g r) -> p g r", r=12),
                        AX.X, OP.add)
                    nc.vector.tensor_add(xs25[:, dc, g:g + 1], tmp[:],
                                         a[ac][:, dc, acol:acol + 1])
        xs[b] = dict(xs2=xs2, xs4=xs4, xs10=xs10, xs25=xs25)

    # ---------------- z / o / B1 ----------------
    zob = [None] * BPC
    for b in range(BPC):
        a = artT[b]
        z_sb = p_zob.tile([128, DC, T], DT, tag="z", name=f"z{b}")
        o_sb = p_zob.tile([128, DC, T], DT, tag="o", name=f"o{b}")
        b1_sb = p_zob.tile([128, DC, T], DT, tag="b1", name=f"b1_{b}")
        for dst, func, bcol, s in ((b1_sb, AF.Relu, 2, 2),
                                   (z_sb, AF.Tanh, 0, 0),
                                   (o_sb, AF.Tanh, 1, 1)):
            for dc in range(DC):
                for t0, tiles in ((0, (0, 1)), (1024, (2, 3))):
                    ps = pp2.tile([128, 1024], F32, tag="zo")
                    c0 = 0
                    for c in tiles:
                        w = TW[c]
                        for kc in range(DC):
                            nc.tensor.matmul(
                                ps[:, c0:c0 + w], w_art_v(kc, s, dc),
                                a[c][:, kc, 0:w],
                                start=(kc == 0), stop=(kc == DC - 1))
                        c0 += w
                    if func == AF.Relu:
                        # relu(x + b) on DVE frees the ACT engine
                        # (gpsimd cannot read PSUM)
                        nc.vector.tensor_scalar(dst[:, dc, t0:t0 + c0],
                                                ps[:, :c0], bias(dc, bcol),
                                                0.0, op0=OP.add, op1=OP.max)
                    else:
                        nc.scalar.activation(dst[:, dc, t0:t0 + c0],
                                             ps[:, :c0], func,
                                             bias=bias(dc, bcol))
        zob[b] = dict(z=z_sb, o=o_sb, b1=b1_sb)

    # ---------------- CE r>=2 (relu on DVE) ----------------
    bls = [None] * BPC
    for b in range(BPC):
        x = xs[b]
        bl = {}
        for ri, (xsr, r) in enumerate(((x["xs2"], 2), (x["xs4"], 4),
                                       (x["xs10"], 10), (x["xs25"], 25))):
            g_r = T // r
            bl[r] = p_xs.tile([128, DC, g_r], DT, tag=f"bl{r}",
                              name=f"bl{r}_{b}")
            for dc in range(DC):
                if g_r > 512:
                    ps = pp2.tile([128, 1024], F32, tag="zo")
                    for half, (g0, gn) in enumerate(((0, 512),
                                                     (512, g_r - 512))):
                        for kc in range(DC):
                            nc.tensor.matmul(
                                ps[:, half * 512:half * 512 + gn],
                                w_ce_v(kc, ri, dc), xsr[:, kc, g0:g0 + gn],
                                start=(kc == 0), stop=(kc == DC - 1))
                    nc.vector.tensor_scalar(bl[r][:, dc, :], ps[:, :g_r],
                                            bias(dc, 3 + ri), 0.0,
                                            op0=OP.add, op1=OP.max)
                else:
                    ps = pp_w.tile([128, 512], F32, tag="w")
                    for kc in range(DC):
                        nc.tensor.matmul(ps[:, :g_r], w_ce_v(kc, ri, dc),
                                         xsr[:, kc, :], start=(kc == 0),
                                         stop=(kc == DC - 1))
                    nc.vector.tensor_scalar(bl[r][:, dc, :], ps[:, :g_r],
                                            bias(dc, 3 + ri), 0.0,
                                            op0=OP.add, op1=OP.max)
        bls[b] = bl

    # ---------------- mix + scan + attention, chunk-pipelined -------------
    # engine round-robin for the mix relus
    _rr = [0]

    def mix_relu(dst, src, bias_ap):
        # gpsimd cannot read PSUM -> alternate ACT (2x) / DVE (1x)
        e = _rr[0] % 3
        _rr[0] += 1
        if e < 2:
            nc.scalar.activation(dst, src, AF.Relu, bias=bias_ap)
        else:
            nc.vector.tensor_scalar(dst, src, bias_ap, 0.0,
                                    op0=OP.add, op1=OP.max)

    gates = [[None] * NSC for _ in range(BPC)]   # gate chunk tiles

    def ev_chunk(b, ri, dc, t0, tn):
        r = RANGES[ri]
        if r == 1:
            return zob[b]["b1"][:, dc, t0:t0 + tn]
        return bls[b][r][:, dc, t0 // r:(t0 + tn) // r, None] \
            .to_broadcast([128, tn // r, r])

    # mix emitted per (b, chunk): for each dc: h1 k=0..2 then gate
    def emit_mix(b, sci):
        t0 = sci * SCW
        gate = p_gate.tile([128, DC, SCW], DT, tag="gate",
                           name=f"gate{b}_{sci}")
        for dc in range(DC):
            h1c = []
            for k in range(3):
                ps = pp_m.tile([128, 512], F32, tag="m")
                for ri in range(5):
                    nc.tensor.matmul(ps[:, :SCW], mI[:, 5 * k + ri, :],
                                     ev_chunk(b, ri, dc, t0, SCW),
                                     start=(ri == 0), stop=(ri == 4))
                h1 = p_h1.tile([128, SCW], DT, tag=f"h1_{k}", name=f"h1_{k}")
                mix_relu(h1[:], ps[:, :SCW], sc(SC_M1B + k))
                h1c.append(h1)
            ps = pp_m.tile([128, 512], F32, tag="m")
            for k in range(3):
                nc.tensor.matmul(ps[:, :SCW], mI[:, 15 + k, :], h1c[k][:],
                                 start=(k == 0), stop=(k == 2))
            mix_relu(gate[:, dc, :], ps[:, :SCW], sc(SC_M2B))
        gates[b][sci] = gate

    # MRU prep + scan + encode for one 500-chunk; engines alternate by dc.
    # The scan result c_t is written back into the z tile (z is dead once
    # (1-g)z is computed), so carry-in for chunk sci is z[:, dc, t0-1].
    def emit_scan(b, sci):
        t0 = sci * SCW
        gate = gates[b][sci]
        z_sb = zob[b]["z"]
        o_sb = zob[b]["o"]
        zz = p_gz.tile([128, DC, SCW], DT, tag="zz", name=f"zz{b}_{sci}")
        for dc in range(DC):
            # TensorTensor muls on gpsimd (SBUF-only engine); the scan
            # itself is a TensorScalarPtr op that only DVE supports.
            zv = z_sb[:, dc, t0:t0 + SCW]
            nc.gpsimd.tensor_mul(zz[:, dc, :], gate[:, dc, :], zv)
            nc.gpsimd.tensor_sub(zz[:, dc, :], zv, zz[:, dc, :])
            init = 0.0 if sci == 0 else z_sb[:, dc, t0 - 1:t0]
            nc.vector.tensor_tensor_scan(zv, gate[:, dc, :], zz[:, dc, :],
                                         init, op0=OP.mult, op1=OP.add)
            # enc chunk: o *= c
            nc.gpsimd.tensor_mul(o_sb[:, dc, t0:t0 + SCW],
                                 o_sb[:, dc, t0:t0 + SCW], zv)

    # attention stream for one 500-chunk
    pbs = [persist.tile([128, 8], F32, tag=f"pb{b}", name=f"pb{b}")
           for b in range(BPC)]

    def emit_attn(b, sci):
        t0 = sci * SCW
        encT = zob[b]["o"]
        s1 = pp_m.tile([TQ, 512], F32, tag="m")
        for dc in range(DC):
            nc.tensor.matmul(s1[:, :SCW], k1T[b][:, dc, :],
                             encT[:, dc, t0:t0 + SCW],
                             start=(dc == 0), stop=(dc == DC - 1))
        e1T = small.tile([TQ, SCW], DT, tag="e1T")
        nc.scalar.activation(e1T[:], s1[:, :SCW], AF.Exp)
        pb_ps = pp_m.tile([128, 512], F32, tag="m", name="pb")
        for si in range(4):
            s0 = si * TSUB
            u2 = pp_w.tile([128, 512], F32, tag="w")
            nc.tensor.matmul(u2[:TSUB, :132], e1T[:, s0:s0 + TSUB],
                             qk_sb[b][:], start=True, stop=True)
            z1 = small.tile([128, 2], F32, tag="z1")
            nc.vector.reciprocal(z1[:TSUB, 1:2], u2[:TSUB, 128:129])
            e2 = small.tile([128, 128], F32, tag="e2")
            nc.scalar.activation(e2[:TSUB, :], u2[:TSUB, 0:128], AF.Exp,
                                 scale=z1[:TSUB, 1:2])
            z2 = small.tile([128, 16], F32, tag="z2")
            nc.vector.tensor_reduce(
                z2[:TSUB, 0:8],
                e2[:TSUB, :].rearrange("p (g w) -> p g w", w=16),
                AX.X, OP.add)
            nc.vector.reciprocal(z2[:TSUB, 8:16], z2[:TSUB, 0:8])
            nc.tensor.matmul(pb_ps[:, :8], e2[:TSUB, :], z2[:TSUB, 8:16],
                             start=(si == 0), stop=(si == 3))
        if sci == 0:
            nc.vector.tensor_copy(pbs[b][:], pb_ps[:, :8])
        else:
            nc.vector.tensor_add(pbs[b][:], pbs[b][:], pb_ps[:, :8])

    # interleaved emission: keep PE dense while scans/attention chase
    emit_mix(0, 0)
    emit_scan(0, 0)
    emit_mix(0, 1)
    emit_scan(0, 1)
    emit_mix(0, 2)
    emit_scan(0, 2)
    emit_mix(0, 3)
    emit_scan(0, 3)
    emit_mix(1, 0)
    emit_attn(0, 0)
    emit_scan(1, 0)
    emit_attn(0, 1)
    emit_mix(1, 1)
    emit_scan(1, 1)
    emit_attn(0, 2)
    emit_mix(1, 2)
    emit_scan(1, 2)
    emit_attn(0, 3)
    emit_mix(1, 3)
    emit_scan(1, 3)
    for sci in range(NSC):
        emit_attn(1, sci)

    # ---------------- answer vectors + final MLP ----------------
    ans_sb = persist.tile([128, 6, 8], F32, tag="ans_sb")
    for b in range(BPC):
        pb_sb = persist.tile([128, 8], DT, tag=f"pbs{b}")
        nc.vector.tensor_copy(pb_sb[:], pbs[b][:])
        pblk = persist.tile([64, 8], DT, tag=f"pblk{b}")
        nc.vector.memset(pblk[:], 0.0)
        for g in range(8):
            o = g % 4
            nc.sync.dma_start(pblk[16 * o:16 * o + 16, g:g + 1],
                              pb_sb[16 * g:16 * g + 16, g:g + 1])
        for dc in range(DC):
            ans_ps = pp_w.tile([128, 512], F32, tag="w")
            nc.tensor.matmul(ans_ps[:, :8],
                             og[b][0:64, 0, dc * DCS:(dc + 1) * DCS],
                             pblk[:], start=True, stop=True)
            # [:, fi*3+dc, b*4:(b+1)*4] <- ans_ps[:, fi*4:(fi+1)*4], 1/T mean
            nc.vector.tensor_scalar_mul(
                ans_sb[:, dc::3, 4 * b:4 * b + 4],
                ans_ps[:, :8].rearrange("p (f o) -> p f o", o=4), 1.0 / T)

    h_ps = pp_w.tile([75, 8], F32, tag="w")
    for j in range(6):
        nc.tensor.matmul(h_ps[:], fp[:, F_AS1 + 75 * j:F_AS1 + 75 * (j + 1)],
                         ans_sb[:, j, :], start=(j == 0), stop=(j == 5))
    h_sb = small.tile([75, 8], F32, tag="h_sb")
    nc.scalar.activation(h_sb[:], h_ps[:], AF.Relu,
                         bias=fp[0:75, F_BAS1:F_BAS1 + 1])
    s_ps = pp_m.tile([128, 512], F32, tag="m", name="s_ps")
    nc.tensor.matmul(s_ps[0:8, 0:1], h_sb[:], fp[0:75, F_AS2:F_AS2 + 1],
                     start=True, stop=True)
    s_sb = small.tile([8, 1], F32, tag="s_sb")
    nc.scalar.activation(s_sb[:], s_ps[0:8, 0:1], AF.Identity,
                         bias=fp[0:8, F_SCAL + SC_AS2B:F_SCAL + SC_AS2B + 1])
    nc.sync.dma_start(out[:].rearrange("b o -> (b o)")[:, None], s_sb[:])


# ---------------------------------------------------------------------------
# host side
# ---------------------------------------------------------------------------

_CACHE = {}


def _get_nc():
    if "nc" not in _CACHE:
        _CACHE["nc"] = _build_program()
    return _CACHE["nc"]


def _pack_weights(inputs):
    f = np.asarray
    wpack = np.zeros((128, WCOLS), np.float32)

    def pad_w(m):  # [300, 300] -> [384, 384]
        p = np.zeros((DPAD, DPAD), np.float32)
        p[:DIM, :DIM] = m
        return p

    w_art = np.zeros((DPAD, 3 * DPAD), np.float32)
    w_art[:DIM, 0 * DPAD:0 * DPAD + DIM] = f(inputs["Wz"]).T
    w_art[:DIM, 1 * DPAD:1 * DPAD + DIM] = f(inputs["Wo"]).T
    w_art[:DIM, 2 * DPAD:2 * DPAD + DIM] = f(inputs["ce_W"])[0].T
    for kc in range(DC):
        rows = slice(kc * DCS, (kc + 1) * DCS)
        wpack[:, W_ART + kc * 1152:W_ART + (kc + 1) * 1152] = w_art[rows]
        for ri in range(4):
            o = W_CE + kc * 1536 + ri * DPAD
            wpack[:, o:o + DPAD] = pad_w(f(inputs["ce_W"])[ri + 1].T)[rows]
        wpack[:, W_F1 + kc * DPAD:W_F1 + (kc + 1) * DPAD] = \
            pad_w(f(inputs["f1_W"]).T)[rows]
        # s2 = aoq @ f2W @ opt^T -> f2/f3 go in untransposed
        wpack[:, W_F2 + kc * DPAD:W_F2 + (kc + 1) * DPAD] = \
            pad_w(f(inputs["f2_W"]))[rows]
        wpack[:, W_F3 + kc * DPAD:W_F3 + (kc + 1) * DPAD] = \
            pad_w(f(inputs["f3_W"]))[rows]

    fpack = np.zeros((128, FCOLS), np.float32)
    biases = np.zeros((DPAD, 10), np.float32)
    biases[:DIM, 0] = f(inputs["bz"])
    biases[:DIM, 1] = f(inputs["bo"])
    for i in range(5):
        biases[:DIM, 2 + i] = f(inputs["ce_b"])[i]
    for kc in range(DC):
        fpack[:, F_BIAS + kc * 10:F_BIAS + (kc + 1) * 10] = \
            biases[kc * DCS:(kc + 1) * DCS]
    m1 = f(inputs["mr1_W"])
    for k in range(3):
        for ri, r in enumerate(RANGES):
            fpack[:, F_SCAL + SC_M1 + 5 * k + ri] = m1[k, ri] / r
    fpack[:, F_SCAL + SC_M1B:F_SCAL + SC_M1B + 3] = f(inputs["mr1_b"])[None, :]
    fpack[:, F_SCAL + SC_M2:F_SCAL + SC_M2 + 3] = f(inputs["mr2_W"])[0][None, :]
    fpack[:, F_SCAL + SC_M2B] = f(inputs["mr2_b"])[0]
    fpack[:, F_SCAL + SC_AS2B] = f(inputs["as2_b"])[0]
    # as1: [75, 600] -> blocks j=fi*3+dc of [128, 75]
    w_as1 = f(inputs["as1_W"])                                # [75, 600]
    for fi in range(2):
        for dc in range(DC):
            j = fi * 3 + dc
            d0 = dc * DCS
            n = min(DCS, DIM - d0) if d0 < DIM else 0
            if n > 0:
                fpack[0:n, F_AS1 + 75 * j:F_AS1 + 75 * (j + 1)] = \
                    w_as1[:, fi * DIM + d0:fi * DIM + d0 + n].T
    fpack[0:75, F_AS2] = f(inputs["as2_W"])[0]
    fpack[0:75, F_BAS1] = f(inputs["as1_b"])
    return wpack.astype(NPDT), fpack


def _wrap16(idx_list):
    """idx i -> [i % 16, i // 16] int16 column block."""
    n = len(idx_list)
    assert n % 16 == 0
    return np.asarray(idx_list, np.int16).reshape(n // 16, 16).T


def _prep_core_inputs(inputs, core):
    b0 = core * BPC
    sl = slice(b0, b0 + BPC)
    f = np.asarray
    if "prep_shared" not in _CACHE:
        wpack, fpack = _pack_weights(inputs)
        _CACHE["prep_shared"] = {
            "wpack": wpack, "fpack": fpack,
            "emb": f(inputs["emb"]).astype(np.float32),
        }
    prep = _CACHE["prep_shared"]

    art = f(inputs["article_in"])[sl].astype(np.int64)
    q = f(inputs["question_in"])[sl].astype(np.int64)
    opts = [f(inputs[f"option{o + 1}_in"])[sl].astype(np.int64)
            for o in range(4)]

    all_tok = np.concatenate([art.ravel(), q.ravel()] +
                             [o.ravel() for o in opts])
    uniq, inv = np.unique(all_tok, return_inverse=True)
    assert len(uniq) <= RMAX, f"{len(uniq)} uniques > {RMAX}"
    embc = np.zeros((RMAX, DPAD), np.float32)
    embc[:len(uniq), :DIM] = prep["emb"][uniq]

    # remapped int16 views in original shapes
    pos = 0
    art_c = inv[pos:pos + art.size].reshape(art.shape); pos += art.size
    q_c = inv[pos:pos + q.size].reshape(q.shape); pos += q.size
    opt_c = []
    for o in range(4):
        opt_c.append(inv[pos:pos + opts[o].size].reshape(opts[o].shape))
        pos += opts[o].size

    ixp = np.zeros((128, IXCOLS), np.int16)

    def put(base, idx_list):
        # idx block must be replicated across all 8 16-partition stripes:
        # each SWDGE queue's Q7 cpu pair reads its own stripe.
        blk = _wrap16(idx_list)
        for c in range(8):
            ixp[16 * c:16 * (c + 1), base:base + blk.shape[1]] = blk

    qopt = np.zeros(256, np.int64)
    qopt[0:TQ] = q_c[0]
    qopt[32:32 + TQ] = q_c[1]
    for b in range(BPC):
        for o in range(4):
            qopt[64 + 64 * b + 16 * o:64 + 64 * b + 16 * (o + 1)] = opt_c[o][b]
    put(IX_QOPT, qopt)
    for b, base in ((0, IX_OG0), (1, IX_OG1)):
        ogl = np.zeros(64, np.int64)
        for o in range(4):
            ogl[16 * o:16 * (o + 1)] = opt_c[o][b]
        put(base, ogl)
    for b, base in ((0, IX_ART0), (1, IX_ART1)):
        al = np.zeros(TA, np.int64)
        al[:T] = art_c[b]
        put(base, al)

    return {
        "embc": embc.astype(NPDT),
        "idx_pack": ixp,
        "wpack": prep["wpack"],
        "fpack": prep["fpack"],
    }


def run_cores(per_core_inputs, trace=False):
    """per_core_inputs: list of 8 dicts name->np array. Returns results."""
    from concourse import bass_utils
    nc = _get_nc()
    return bass_utils.run_bass_kernel_spmd(
        nc, per_core_inputs, core_ids=list(range(NCORES)),
        trace=trace, trace_cores=[0] if trace else None)


def kernel(**inputs):
    _CACHE.pop("prep_shared", None)
    per_core = [_prep_core_inputs(inputs, c) for c in range(NCORES)]
    res = run_cores(per_core)
    out = np.concatenate([res.results[c]["scores"] for c in range(NCORES)],
                         axis=0)
    return out.astype(np.float32)


# revision 20
# speedup vs baseline: 1.1504x; 1.1504x over previous
"""BiAttentionMRU Trainium2 kernel.

Data-parallel over batch: B=16 -> 2 batch elements on each of 8 cores.
All weights replicated; the embedding is host-compacted to each core's
working set (~4k unique tokens) so the on-device gather can use the
batched SWDGE dma_gather in TRANSPOSE mode, which lands the article
directly in [d-on-partitions, token-cols] layout (no PE transposes) and
costs ~1us of gpsimd issue per 2048 tokens instead of ~17us.

Layouts: everything is [d, t] with d padded 300->384 = 3 chunks of 128
(pad rows are zeros end-to-end: emb pad cols, weight pad rows/cols and
bias pad rows are all zero, so pad lanes carry exact zeros through
z/o/gate/scan/attention).

Pipeline: a burst of zero-weight dummy matmuls at t=0 trips the PE HAM
clock gate to 2.4GHz before the real GEMMs arrive; z/o/B1 + CE stream
per batch as the gathers land; the 5->3->1 gate mix runs as
scaled-identity accumulating matmuls in 500-col chunks whose relus are
round-robined over Scalar/Vector/GpSimd; the MRU scan runs in 500-col
carry-chained chunks that chase the gate chunks, and the attention
(same exp/Z-folding algebra as before, 500-col chunks) chases the scan,
interleaved across the two batch elements to keep PE dense.
"""

import sys

sys.path.insert(0, "/opt/trn_rl_repo")

import numpy as np
import ml_dtypes

import concourse.bass as bass
import concourse.tile as tile
from concourse import bacc, mybir
from concourse.masks import make_identity

F32 = mybir.dt.float32
BF16 = mybir.dt.bfloat16
I16 = mybir.dt.int16
AX = mybir.AxisListType
OP = mybir.AluOpType
AF = mybir.ActivationFunctionType

DIM = 300
DPAD = 384
B_FULL = 16
NCORES = 8
BPC = B_FULL // NCORES  # batch per core = 2
T = 2000
TA = 2048               # article gather length (48 pad idx-0 tokens)
TQ = 30
TO = 16
RANGES = (1, 2, 4, 10, 25)
RMAX = 4608             # compacted per-core vocab rows (>= ~4.1k uniques)

DCS = 128
DC = 3

DT = BF16
NPDT = ml_dtypes.bfloat16

# z/o/B1 psum pairs (matmuls at <=512 cols, one ACT per 2-bank pair)
TP = [(0, 1024, (512, 512)), (1024, 976, (512, 464))]
# mix / scan / attention 500-col chunks
NSC = 4
SCW = 500
TSUB = 125  # attention sub-chunk (partitions of the s2 block)

# ---- packed bf16 weights: [128, 11520] ----
W_ART = 0            # 3 kc * (z|o|ce0) * 384
W_CE = 3456          # 3 kc * 4 ri * 384
W_F1 = 8064          # 3 kc * 384 (transposed)
W_F2 = 9216          # untransposed
W_F3 = 10368
WCOLS = 11520

# ---- packed f32 tensor: [128, 506] ----
F_BIAS = 0           # dc*10 + {0 bz, 1 bo, 2..6 ce_b[0..4]}
F_SCAL = 30          # 24 scalar cols (below)
F_AS1 = 54           # 6 blocks of 75 (block j = fi*3+dc)
F_AS2 = 504          # rows 0..74
F_BAS1 = 505         # rows 0..74
FCOLS = 506

SC_M1 = 0            # 15 cols: m1[k,r]/r at 5k+ri
SC_M1B = 15          # 3 cols: mr1_b
SC_M2 = 18           # 3 cols: mr2_W
SC_M2B = 21          # mr2_b
SC_AS2B = 22         # as2_b

# ---- packed i16 indices: [128, 280], idx i of a list at [i%16, base+i//16]
IX_QOPT = 0          # 256: q_b0(30+2), q_b1(30+2), opt_b0(64), opt_b1(64), pad
IX_OG0 = 16          # 64: b0 options, row layout
IX_OG1 = 20          # 64: b1 options
IX_ART0 = 24         # 2048: b0 article (+48 pad)
IX_ART1 = 152        # 2048: b1 article
IXCOLS = 280

N_WARM = 36          # dummy matmuls to trip the HAM clock gate at t=0


def _build_program():
    nc = bacc.Bacc("TRN2", target_bir_lowering=False, debug=False,
                   num_devices=NCORES, num_swdge_queues=4)

    embc = nc.dram_tensor("embc", [RMAX, DPAD], DT, kind="ExternalInput")
    idx_pack = nc.dram_tensor("idx_pack", [128, IXCOLS], I16,
                              kind="ExternalInput")
    wpack = nc.dram_tensor("wpack", [128, WCOLS], DT, kind="ExternalInput")
    fpack = nc.dram_tensor("fpack", [128, FCOLS], F32, kind="ExternalInput")
    out = nc.dram_tensor("scores", [BPC, 4], F32, kind="ExternalOutput")

    with tile.TileContext(nc) as tc:
        from contextlib import ExitStack
        with ExitStack() as ctx:
            _emit(nc, tc, ctx, embc, idx_pack, wpack, fpack, out)

    nc.compile()
    return nc


def _emit(nc, tc, ctx, embc, idx_pack, wpack, fpack, out):
    # ---------------- pools ----------------
    consts = ctx.enter_context(tc.tile_pool(name="consts", bufs=1))
    persist = ctx.enter_context(tc.tile_pool(name="persist", bufs=1))
    p_art = ctx.enter_context(tc.tile_pool(name="p_art", bufs=8))
    p_zob = ctx.enter_context(tc.tile_pool(name="p_zob", bufs=2))
    p_xs = ctx.enter_context(tc.tile_pool(name="p_xs", bufs=2))
    p_h1 = ctx.enter_context(tc.tile_pool(name="p_h1", bufs=2))
    p_gate = ctx.enter_context(tc.tile_pool(name="p_gate", bufs=3))
    p_gz = ctx.enter_context(tc.tile_pool(name="p_gz", bufs=2))
    small = ctx.enter_context(tc.tile_pool(name="small", bufs=2))
    # PSUM (8 banks): pp2 2x2-bank (zob/CE-r2/pb) + mix 2x1 + work 2x1
    pp2 = ctx.enter_context(tc.tile_pool(name="pp2", bufs=2, space="PSUM"))
    pp_m = ctx.enter_context(tc.tile_pool(name="pp_m", bufs=2, space="PSUM"))
    pp_w = ctx.enter_context(tc.tile_pool(name="pp_w", bufs=2, space="PSUM"))

    # ---------------- HAM warm-up: dummy matmuls on zeroed tiles ----------
    wz_l = consts.tile([128, 128], DT)
    wz_r = consts.tile([128, 512], DT)
    nc.vector.memset(wz_l[:], 0.0)
    nc.vector.memset(wz_r[:], 0.0)
    for _ in range(N_WARM):
        ps = pp_w.tile([128, 512], F32, tag="w")
        nc.tensor.matmul(ps[:], wz_l[:], wz_r[:], start=True, stop=True)

    # ---------------- index pack + gathers ----------------
    ixp = consts.tile([128, IXCOLS], I16)
    nc.sync.dma_start(ixp[:], idx_pack[:])

    def gather(out_ap, ixcol, n, transpose, q):
        nc.gpsimd.dma_gather(
            out_ap, embc[:], ixp[:, ixcol:ixcol + (n + 15) // 16],
            n, n, DPAD, transpose=transpose, queue_num=q)

    # queue_num must track the DMASW round-robin (emission order % 4) so
    # Tile's per-queue semaphore binding stays consistent
    qoptT = persist.tile([128, DC, 256], DT, tag="qoptT")
    gather(qoptT[:], IX_QOPT, 256, True, 0)
    og = [persist.tile([128, 1, DPAD], DT, tag=f"og{b}", name=f"og{b}")
          for b in range(BPC)]
    gather(og[0][:], IX_OG0, 64, False, 1)
    gather(og[1][:], IX_OG1, 64, False, 2)
    # article in 512-token chunks: one transpose gather pushes one tx
    # descriptor per index and the SWDGE ring wedges above ~512
    artT = [[p_art.tile([128, DC, 512], DT, tag="artT", name=f"artT{b}_{c}")
             for c in range(4)] for b in range(BPC)]
    gi = 3
    for b, base in ((0, IX_ART0), (1, IX_ART1)):
        for c in range(4):
            gather(artT[b][c][:], base + 32 * c, 512, True, gi % 4)
            gi += 1

    # q/opt transposed views (cols within qoptT)
    def qT(b):           # [128, DC, 30]
        return qoptT[:, :, 32 * b:32 * b + TQ]

    def oT(b):           # [128, DC, 64] = (o w)
        return qoptT[:, :, 64 + 64 * b:128 + 64 * b]

    # ---------------- weights ----------------
    wp = consts.tile([128, WCOLS], DT)
    nc.sync.dma_start(wp[:], wpack[:])
    fp = consts.tile([128, FCOLS], F32)
    nc.sync.dma_start(fp[:], fpack[:])

    def w_art_v(kc, s, dc):
        o = W_ART + kc * 1152 + s * DPAD + dc * DCS
        return wp[:, o:o + DCS]

    def w_ce_v(kc, ri, dc):
        o = W_CE + kc * 1536 + ri * DPAD + dc * DCS
        return wp[:, o:o + DCS]

    def w_f_v(base, kc):
        return wp[:, base + kc * DPAD:base + (kc + 1) * DPAD]

    def bias(dc, col):
        return fp[:, dc * 10 + col:dc * 10 + col + 1]

    def sc(col):
        return fp[:, F_SCAL + col:F_SCAL + col + 1]

    ident = consts.tile([128, 128], DT)
    make_identity(nc, ident[:])

    # scaled identities for the PE-side gate mix
    mI = consts.tile([128, 18, 128], DT)
    for j in range(18):
        scol = (SC_M1 + j) if j < 15 else (SC_M2 + j - 15)
        nc.vector.tensor_scalar_mul(mI[:, j, :], ident[:], sc(scol))

    # ---------------- attention prep (needs only qoptT) ----------------
    k1T = [persist.tile([128, DC, TQ], DT, tag=f"k1T{b}", name=f"k1T{b}")
           for b in range(BPC)]
    qk_sb = [persist.tile([TQ, 132], DT, tag=f"qk{b}", name=f"qk{b}")
             for b in range(BPC)]
    for b in range(BPC):
        for dc in range(DC):
            ps = pp_w.tile([128, 512], F32, tag="w")
            for kc in range(DC):
                nc.tensor.matmul(ps[:, :TQ],
                                 w_f_v(W_F1, kc)[:, dc * DCS:(dc + 1) * DCS],
                                 qT(b)[:, kc, :], start=(kc == 0),
                                 stop=(kc == DC - 1))
            nc.scalar.copy(k1T[b][:, dc, :], ps[:, :TQ])

        aTs = []
        for fi, base in enumerate((W_F2, W_F3)):
            a_ps = pp_w.tile([TQ, DPAD], F32, tag="w")
            for kc in range(DC):
                nc.tensor.matmul(a_ps[:], qT(b)[:, kc, :], w_f_v(base, kc),
                                 start=(kc == 0), stop=(kc == DC - 1))
            a_sb = small.tile([TQ, DPAD], DT, tag="a_sb")
            nc.vector.tensor_copy(a_sb[:], a_ps[:])
            aT = persist.tile([128, DC, TQ], DT, tag=f"aT{fi}_{b}", name=f"aT{fi}_{b}")
            for dc in range(DC):
                tp = pp_w.tile([128, 512], DT, tag="w")
                nc.tensor.transpose(tp[:, :TQ],
                                    a_sb[:, dc * DCS:(dc + 1) * DCS],
                                    ident[:TQ, :TQ])
                nc.vector.tensor_copy(aT[:, dc, :], tp[:, :TQ])
            aTs.append(aT)

        qk_ps = pp_w.tile([TQ, 512], F32, tag="w")
        for fi in range(2):
            for kc in range(DC):
                nc.tensor.matmul(qk_ps[:, 64 * fi:64 * fi + 64],
                                 aTs[fi][:, kc, :], oT(b)[:, kc, :],
                                 start=(kc == 0), stop=(kc == DC - 1))
        nc.vector.tensor_copy(qk_sb[b][:, 0:128], qk_ps[:, :128])
        nc.vector.memset(qk_sb[b][:, 128:132], 1.0)

    # ---------------- group sums (xs_r in [d, g]) ----------------
    # xs2 per 512-tile (pairs are 2-aligned); xs4/xs10 from xs2; xs25 full
    # groups per tile + 3 straddle groups patched from xs2 plus one article
    # column (25g odd/even cases worked out per straddle).
    TW = [512, 512, 512, 464]
    xs = [None] * BPC
    for b in range(BPC):
        a = artT[b]
        xs2 = p_xs.tile([128, DC, T // 2], DT, tag="xs2", name=f"xs2_{b}")
        xs4 = p_xs.tile([128, DC, T // 4], DT, tag="xs4", name=f"xs4_{b}")
        xs10 = p_xs.tile([128, DC, T // 10], DT, tag="xs10", name=f"xs10_{b}")
        xs25 = p_xs.tile([128, DC, T // 25], DT, tag="xs25", name=f"xs25_{b}")
        with nc.allow_low_precision(reason="bf16 group sums"):
            for dc in range(DC):
                for c in range(4):
                    w = TW[c]
                    nc.vector.tensor_add(
                        xs2[:, dc, 256 * c:256 * c + w // 2],
                        a[c][:, dc, 0:w:2], a[c][:, dc, 1:w:2])
                    t0 = 512 * c
                    gs, ge = -(-t0 // 25), (t0 + w) // 25
                    nc.vector.tensor_reduce(
                        xs25[:, dc, gs:ge],
                        a[c][:, dc, 25 * gs - t0:25 * ge - t0].rearrange(
                            "p (g r) -> p g r", r=25),
                        AX.X, OP.add)
                for r0 in range(0, T, 500):
                    h0, h1r = r0 // 2, (r0 + 500) // 2
                    nc.gpsimd.tensor_add(xs4[:, dc, r0 // 4:(r0 + 500) // 4],
                                         xs2[:, dc, h0:h1r:2],
                                         xs2[:, dc, h0 + 1:h1r:2])
                    nc.vector.tensor_reduce(
                        xs10[:, dc, r0 // 10:(r0 + 500) // 10],
                        xs2[:, dc, h0:h1r].rearrange("p (g r) -> p g r", r=5),
                        AX.X, OP.add)
                # straddle groups: (g, xs2 col range, art tile, art col)
                for g, x0, ac, acol in ((20, 250, 1, 12), (40, 500, 2, 0),
                                        (61, 763, 2, 501)):
                    tmp = small.tile([128, 1], DT, tag="s25", name="s25")
                    nc.vector.tensor_reduce(
                        tmp[:, :],
                        xs2[:, dc, x0:x0 + 12].rearrange(
                            "p (g r) -> p g r", r=12),
                        AX.X, OP.add)
                    nc.vector.tensor_add(xs25[:, dc, g:g + 1], tmp[:],
                                         a[ac][:, dc, acol:acol + 1])
        xs[b] = dict(xs2=xs2, xs4=xs4, xs10=xs10, xs25=xs25)

    # ---------------- z / o / B1 ----------------
    zob = [None] * BPC
    for b in range(BPC):
        a = artT[b]
        z_sb = p_zob.tile([128, DC, T], DT, tag="z", name=f"z{b}")
        o_sb = p_zob.tile([128, DC, T], DT, tag="o", name=f"o{b}")
        b1_sb = p_zob.tile([128, DC, T], DT, tag="b1", name=f"b1_{b}")
        for dst, func, bcol, s in ((b1_sb, AF.Relu, 2, 2),
                                   (z_sb, AF.Tanh, 0, 0),
                                   (o_sb, AF.Tanh, 1, 1)):
            for dc in range(DC):
                for t0, tiles in ((0, (0, 1)), (1024, (2, 3))):
                    ps = pp2.tile([128, 1024], F32, tag="zo")
                    c0 = 0
                    for c in tiles:
                        w = TW[c]
                        for kc in range(DC):
                            nc.tensor.matmul(
                                ps[:, c0:c0 + w], w_art_v(kc, s, dc),
                                a[c][:, kc, 0:w],
                                start=(kc == 0), stop=(kc == DC - 1))
                        c0 += w
                    if func == AF.Relu:
                        # relu(x + b) on DVE frees the ACT engine
                        # (gpsimd cannot read PSUM)
                        nc.vector.tensor_scalar(dst[:, dc, t0:t0 + c0],
                                                ps[:, :c0], bias(dc, bcol),
                                                0.0, op0=OP.add, op1=OP.max)
                    else:
                        nc.scalar.activation(dst[:, dc, t0:t0 + c0],
                                             ps[:, :c0], func,
                                             bias=bias(dc, bcol))
        zob[b] = dict(z=z_sb, o=o_sb, b1=b1_sb)

    # ---------------- CE r>=2 (relu on DVE) ----------------
    bls = [None] * BPC
    for b in range(BPC):
        x = xs[b]
        bl = {}
        for ri, (xsr, r) in enumerate(((x["xs2"], 2), (x["xs4"], 4),
                                       (x["xs10"], 10), (x["xs25"], 25))):
            g_r = T // r
            bl[r] = p_xs.tile([128, DC, g_r], DT, tag=f"bl{r}",
                              name=f"bl{r}_{b}")
            for dc in range(DC):
                if g_r > 512:
                    ps = pp2.tile([128, 1024], F32, tag="zo")
                    for half, (g0, gn) in enumerate(((0, 512),
                                                     (512, g_r - 512))):
                        for kc in range(DC):
                            nc.tensor.matmul(
                                ps[:, half * 512:half * 512 + gn],
                                w_ce_v(kc, ri, dc), xsr[:, kc, g0:g0 + gn],
                                start=(kc == 0), stop=(kc == DC - 1))
                    nc.scalar.activation(bl[r][:, dc, :], ps[:, :g_r],
                                         AF.Relu, bias=bias(dc, 3 + ri))
                else:
                    ps = pp_w.tile([128, 512], F32, tag="w")
                    for kc in range(DC):
                        nc.tensor.matmul(ps[:, :g_r], w_ce_v(kc, ri, dc),
                                         xsr[:, kc, :], start=(kc == 0),
                                         stop=(kc == DC - 1))
                    nc.scalar.activation(bl[r][:, dc, :], ps[:, :g_r],
                                         AF.Relu, bias=bias(dc, 3 + ri))
        bls[b] = bl

    # ---------------- mix + scan + attention, chunk-pipelined -------------
    # engine round-robin for the mix relus
    _rr = [0]

    def mix_relu(dst, src, bias_ap):
        # gpsimd cannot read PSUM -> alternate ACT (2x) / DVE (1x)
        e = _rr[0] % 3
        _rr[0] += 1
        if e < 2:
            nc.scalar.activation(dst, src, AF.Relu, bias=bias_ap)
        else:
            nc.vector.tensor_scalar(dst, src, bias_ap, 0.0,
                                    op0=OP.add, op1=OP.max)

    gates = [[None] * NSC for _ in range(BPC)]   # gate chunk tiles

    def ev_chunk(b, ri, dc, t0, tn):
        r = RANGES[ri]
        if r == 1:
            return zob[b]["b1"][:, dc, t0:t0 + tn]
        return bls[b][r][:, dc, t0 // r:(t0 + tn) // r, None] \
            .to_broadcast([128, tn // r, r])

    # mix emitted per (b, chunk): for each dc: h1 k=0..2 then gate
    def emit_mix(b, sci):
        t0 = sci * SCW
        gate = p_gate.tile([128, DC, SCW], DT, tag="gate",
                           name=f"gate{b}_{sci}")
        for dc in range(DC):
            h1c = []
            for k in range(3):
                ps = pp_m.tile([128, 512], F32, tag="m")
                for ri in range(5):
                    nc.tensor.matmul(ps[:, :SCW], mI[:, 5 * k + ri, :],
                                     ev_chunk(b, ri, dc, t0, SCW),
                                     start=(ri == 0), stop=(ri == 4))
                h1 = p_h1.tile([128, SCW], DT, tag=f"h1_{k}", name=f"h1_{k}")
                mix_relu(h1[:], ps[:, :SCW], sc(SC_M1B + k))
                h1c.append(h1)
            ps = pp_m.tile([128, 512], F32, tag="m")
            for k in range(3):
                nc.tensor.matmul(ps[:, :SCW], mI[:, 15 + k, :], h1c[k][:],
                                 start=(k == 0), stop=(k == 2))
            mix_relu(gate[:, dc, :], ps[:, :SCW], sc(SC_M2B))
        gates[b][sci] = gate

    # MRU prep + scan + encode for one 500-chunk; engines alternate by dc.
    # The scan result c_t is written back into the z tile (z is dead once
    # (1-g)z is computed), so carry-in for chunk sci is z[:, dc, t0-1].
    def emit_scan(b, sci):
        t0 = sci * SCW
        gate = gates[b][sci]
        z_sb = zob[b]["z"]
        o_sb = zob[b]["o"]
        zz = p_gz.tile([128, DC, SCW], DT, tag="zz", name=f"zz{b}_{sci}")
        for dc in range(DC):
            # TensorTensor muls on gpsimd (SBUF-only engine); the scan
            # itself is a TensorScalarPtr op that only DVE supports.
            zv = z_sb[:, dc, t0:t0 + SCW]
            nc.vector.tensor_mul(zz[:, dc, :], gate[:, dc, :], zv)
            nc.vector.tensor_sub(zz[:, dc, :], zv, zz[:, dc, :])
            init = 0.0 if sci == 0 else z_sb[:, dc, t0 - 1:t0]
            nc.vector.tensor_tensor_scan(zv, gate[:, dc, :], zz[:, dc, :],
                                         init, op0=OP.mult, op1=OP.add)
            # enc chunk: o *= c  (gpsimd: SBUF-only TT, keeps DVE free)
            nc.gpsimd.tensor_mul(o_sb[:, dc, t0:t0 + SCW],
                                 o_sb[:, dc, t0:t0 + SCW], zv)

    # attention stream for one 500-chunk
    pbs = [persist.tile([128, 8], F32, tag=f"pb{b}", name=f"pb{b}")
           for b in range(BPC)]

    def emit_attn(b, sci):
        t0 = sci * SCW
        encT = zob[b]["o"]
        s1 = pp_m.tile([TQ, 512], F32, tag="m")
        for dc in range(DC):
            nc.tensor.matmul(s1[:, :SCW], k1T[b][:, dc, :],
                             encT[:, dc, t0:t0 + SCW],
                             start=(dc == 0), stop=(dc == DC - 1))
        e1T = small.tile([TQ, SCW], DT, tag="e1T")
        nc.scalar.activation(e1T[:], s1[:, :SCW], AF.Exp)
        pb_ps = pp_m.tile([128, 512], F32, tag="m", name="pb")
        for si in range(4):
            s0 = si * TSUB
            u2 = pp_w.tile([128, 512], F32, tag="w")
            nc.tensor.matmul(u2[:TSUB, :132], e1T[:, s0:s0 + TSUB],
                             qk_sb[b][:], start=True, stop=True)
            z1 = small.tile([128, 2], F32, tag="z1")
            nc.vector.reciprocal(z1[:TSUB, 1:2], u2[:TSUB, 128:129])
            e2 = small.tile([128, 128], F32, tag="e2")
            nc.scalar.activation(e2[:TSUB, :], u2[:TSUB, 0:128], AF.Exp,
                                 scale=z1[:TSUB, 1:2])
            z2 = small.tile([128, 16], F32, tag="z2")
            nc.vector.tensor_reduce(
                z2[:TSUB, 0:8],
                e2[:TSUB, :].rearrange("p (g w) -> p g w", w=16),
                AX.X, OP.add)
            nc.vector.reciprocal(z2[:TSUB, 8:16], z2[:TSUB, 0:8])
            nc.tensor.matmul(pb_ps[:, :8], e2[:TSUB, :], z2[:TSUB, 8:16],
                             start=(si == 0), stop=(si == 3))
        if sci == 0:
            nc.vector.tensor_copy(pbs[b][:], pb_ps[:, :8])
        else:
            nc.vector.tensor_add(pbs[b][:], pbs[b][:], pb_ps[:, :8])

    # interleaved emission: keep PE dense while scans/attention chase
    emit_mix(0, 0)
    emit_scan(0, 0)
    emit_mix(0, 1)
    emit_scan(0, 1)
    emit_mix(0, 2)
    emit_scan(0, 2)
    emit_mix(0, 3)
    emit_scan(0, 3)
    emit_mix(1, 0)
    emit_attn(0, 0)
    emit_scan(1, 0)
    emit_attn(0, 1)
    emit_mix(1, 1)
    emit_scan(1, 1)
    emit_attn(0, 2)
    emit_mix(1, 2)
    emit_scan(1, 2)
    emit_attn(0, 3)
    emit_mix(1, 3)
    emit_scan(1, 3)
    for sci in range(NSC):
        emit_attn(1, sci)

    # ---------------- answer vectors + final MLP ----------------
    ans_sb = persist.tile([128, 6, 8], F32, tag="ans_sb")
    for b in range(BPC):
        pb_sb = persist.tile([128, 8], DT, tag=f"pbs{b}")
        nc.vector.tensor_copy(pb_sb[:], pbs[b][:])
        pblk = persist.tile([64, 8], DT, tag=f"pblk{b}")
        nc.vector.memset(pblk[:], 0.0)
        for g in range(8):
            o = g % 4
            nc.sync.dma_start(pblk[16 * o:16 * o + 16, g:g + 1],
                              pb_sb[16 * g:16 * g + 16, g:g + 1])
        for dc in range(DC):
            ans_ps = pp_w.tile([128, 512], F32, tag="w")
            nc.tensor.matmul(ans_ps[:, :8],
                             og[b][0:64, 0, dc * DCS:(dc + 1) * DCS],
                             pblk[:], start=True, stop=True)
            # [:, fi*3+dc, b*4:(b+1)*4] <- ans_ps[:, fi*4:(fi+1)*4], 1/T mean
            nc.vector.tensor_scalar_mul(
                ans_sb[:, dc::3, 4 * b:4 * b + 4],
                ans_ps[:, :8].rearrange("p (f o) -> p f o", o=4), 1.0 / T)

    h_ps = pp_w.tile([75, 8], F32, tag="w")
    for j in range(6):
        nc.tensor.matmul(h_ps[:], fp[:, F_AS1 + 75 * j:F_AS1 + 75 * (j + 1)],
                         ans_sb[:, j, :], start=(j == 0), stop=(j == 5))
    h_sb = small.tile([75, 8], F32, tag="h_sb")
    nc.scalar.activation(h_sb[:], h_ps[:], AF.Relu,
                         bias=fp[0:75, F_BAS1:F_BAS1 + 1])
    s_ps = pp_m.tile([128, 512], F32, tag="m", name="s_ps")
    nc.tensor.matmul(s_ps[0:8, 0:1], h_sb[:], fp[0:75, F_AS2:F_AS2 + 1],
                     start=True, stop=True)
    s_sb = small.tile([8, 1], F32, tag="s_sb")
    nc.scalar.activation(s_sb[:], s_ps[0:8, 0:1], AF.Identity,
                         bias=fp[0:8, F_SCAL + SC_AS2B:F_SCAL + SC_AS2B + 1])
    nc.sync.dma_start(out[:].rearrange("b o -> (b o)")[:, None], s_sb[:])


# ---------------------------------------------------------------------------
# host side
# ---------------------------------------------------------------------------

_CACHE = {}


def _get_nc():
    if "nc" not in _CACHE:
        _CACHE["nc"] = _build_program()
    return _CACHE["nc"]


def _pack_weights(inputs):
    f = np.asarray
    wpack = np.zeros((128, WCOLS), np.float32)

    def pad_w(m):  # [300, 300] -> [384, 384]
        p = np.zeros((DPAD, DPAD), np.float32)
        p[:DIM, :DIM] = m
        return p

    w_art = np.zeros((DPAD, 3 * DPAD), np.float32)
    w_art[:DIM, 0 * DPAD:0 * DPAD + DIM] = f(inputs["Wz"]).T
    w_art[:DIM, 1 * DPAD:1 * DPAD + DIM] = f(inputs["Wo"]).T
    w_art[:DIM, 2 * DPAD:2 * DPAD + DIM] = f(inputs["ce_W"])[0].T
    for kc in range(DC):
        rows = slice(kc * DCS, (kc + 1) * DCS)
        wpack[:, W_ART + kc * 1152:W_ART + (kc + 1) * 1152] = w_art[rows]
        for ri in range(4):
            o = W_CE + kc * 1536 + ri * DPAD
            wpack[:, o:o + DPAD] = pad_w(f(inputs["ce_W"])[ri + 1].T)[rows]
        wpack[:, W_F1 + kc * DPAD:W_F1 + (kc + 1) * DPAD] = \
            pad_w(f(inputs["f1_W"]).T)[rows]
        # s2 = aoq @ f2W @ opt^T -> f2/f3 go in untransposed
        wpack[:, W_F2 + kc * DPAD:W_F2 + (kc + 1) * DPAD] = \
            pad_w(f(inputs["f2_W"]))[rows]
        wpack[:, W_F3 + kc * DPAD:W_F3 + (kc + 1) * DPAD] = \
            pad_w(f(inputs["f3_W"]))[rows]

    fpack = np.zeros((128, FCOLS), np.float32)
    biases = np.zeros((DPAD, 10), np.float32)
    biases[:DIM, 0] = f(inputs["bz"])
    biases[:DIM, 1] = f(inputs["bo"])
    for i in range(5):
        biases[:DIM, 2 + i] = f(inputs["ce_b"])[i]
    for kc in range(DC):
        fpack[:, F_BIAS + kc * 10:F_BIAS + (kc + 1) * 10] = \
            biases[kc * DCS:(kc + 1) * DCS]
    m1 = f(inputs["mr1_W"])
    for k in range(3):
        for ri, r in enumerate(RANGES):
            fpack[:, F_SCAL + SC_M1 + 5 * k + ri] = m1[k, ri] / r
    fpack[:, F_SCAL + SC_M1B:F_SCAL + SC_M1B + 3] = f(inputs["mr1_b"])[None, :]
    fpack[:, F_SCAL + SC_M2:F_SCAL + SC_M2 + 3] = f(inputs["mr2_W"])[0][None, :]
    fpack[:, F_SCAL + SC_M2B] = f(inputs["mr2_b"])[0]
    fpack[:, F_SCAL + SC_AS2B] = f(inputs["as2_b"])[0]
    # as1: [75, 600] -> blocks j=fi*3+dc of [128, 75]
    w_as1 = f(inputs["as1_W"])                                # [75, 600]
    for fi in range(2):
        for dc in range(DC):
            j = fi * 3 + dc
            d0 = dc * DCS
            n = min(DCS, DIM - d0) if d0 < DIM else 0
            if n > 0:
                fpack[0:n, F_AS1 + 75 * j:F_AS1 + 75 * (j + 1)] = \
                    w_as1[:, fi * DIM + d0:fi * DIM + d0 + n].T
    fpack[0:75, F_AS2] = f(inputs["as2_W"])[0]
    fpack[0:75, F_BAS1] = f(inputs["as1_b"])
    return wpack.astype(NPDT), fpack


def _wrap16(idx_list):
    """idx i -> [i % 16, i // 16] int16 column block."""
    n = len(idx_list)
    assert n % 16 == 0
    return np.asarray(idx_list, np.int16).reshape(n // 16, 16).T


def _prep_core_inputs(inputs, core):
    b0 = core * BPC
    sl = slice(b0, b0 + BPC)
    f = np.asarray
    if "prep_shared" not in _CACHE:
        wpack, fpack = _pack_weights(inputs)
        _CACHE["prep_shared"] = {
            "wpack": wpack, "fpack": fpack,
            "emb": f(inputs["emb"]).astype(np.float32),
        }
    prep = _CACHE["prep_shared"]

    art = f(inputs["article_in"])[sl].astype(np.int64)
    q = f(inputs["question_in"])[sl].astype(np.int64)
    opts = [f(inputs[f"option{o + 1}_in"])[sl].astype(np.int64)
            for o in range(4)]

    all_tok = np.concatenate([art.ravel(), q.ravel()] +
                             [o.ravel() for o in opts])
    uniq, inv = np.unique(all_tok, return_inverse=True)
    assert len(uniq) <= RMAX, f"{len(uniq)} uniques > {RMAX}"
    embc = np.zeros((RMAX, DPAD), np.float32)
    embc[:len(uniq), :DIM] = prep["emb"][uniq]

    # remapped int16 views in original shapes
    pos = 0
    art_c = inv[pos:pos + art.size].reshape(art.shape); pos += art.size
    q_c = inv[pos:pos + q.size].reshape(q.shape); pos += q.size
    opt_c = []
    for o in range(4):
        opt_c.append(inv[pos:pos + opts[o].size].reshape(opts[o].shape))
        pos += opts[o].size

    ixp = np.zeros((128, IXCOLS), np.int16)

    def put(base, idx_list):
        # idx block must be replicated across all 8 16-partition stripes:
        # each SWDGE queue's Q7 cpu pair reads its own stripe.
        blk = _wrap16(idx_list)
        for c in range(8):
            ixp[16 * c:16 * (c + 1), base:base + blk.shape[1]] = blk

    qopt = np.zeros(256, np.int64)
    qopt[0:TQ] = q_c[0]
    qopt[32:32 + TQ] = q_c[1]
    for b in range(BPC):
        for o in range(4):
            qopt[64 + 64 * b + 16 * o:64 + 64 * b + 16 * (o + 1)] = opt_c[o][b]
    put(IX_QOPT, qopt)
    for b, base in ((0, IX_OG0), (1, IX_OG1)):
        ogl = np.zeros(64, np.int64)
        for o in range(4):
            ogl[16 * o:16 * (o + 1)] = opt_c[o][b]
        put(base, ogl)
    for b, base in ((0, IX_ART0), (1, IX_ART1)):
        al = np.zeros(TA, np.int64)
        al[:T] = art_c[b]
        put(base, al)

    return {
        "embc": embc.astype(NPDT),
        "idx_pack": ixp,
        "wpack": prep["wpack"],
        "fpack": prep["fpack"],
    }


def run_cores(per_core_inputs, trace=False):
    """per_core_inputs: list of 8 dicts name->np array. Returns results."""
    from concourse import bass_utils
    nc = _get_nc()
    return bass_utils.run_bass_kernel_spmd(
        nc, per_core_inputs, core_ids=list(range(NCORES)),
        trace=trace, trace_cores=[0] if trace else None)


def kernel(**inputs):
    _CACHE.pop("prep_shared", None)
    per_core = [_prep_core_inputs(inputs, c) for c in range(NCORES)]
    res = run_cores(per_core)
    out = np.concatenate([res.results[c]["scores"] for c in range(NCORES)],
                         axis=0)
    return out.astype(np.float32)


# revision 23
# speedup vs baseline: 1.2674x; 1.1017x over previous
"""BiAttentionMRU Trainium2 kernel.

Data-parallel over batch: B=16 -> 2 batch elements on each of 8 cores.
All weights replicated; the embedding is host-compacted to each core's
working set (~4k unique tokens) so the on-device gather can use the
batched SWDGE dma_gather in TRANSPOSE mode, which lands the article
directly in [d-on-partitions, token-cols] layout (no PE transposes) and
costs ~1us of gpsimd issue per 2048 tokens instead of ~17us.

Layouts: everything is [d, t] with d padded 300->384 = 3 chunks of 128
(pad rows are zeros end-to-end: emb pad cols, weight pad rows/cols and
bias pad rows are all zero, so pad lanes carry exact zeros through
z/o/gate/scan/attention).

Pipeline: a burst of zero-weight dummy matmuls at t=0 trips the PE HAM
clock gate to 2.4GHz before the real GEMMs arrive; z/o/B1 + CE stream
per batch as the gathers land; the 5->3->1 gate mix runs as
scaled-identity accumulating matmuls in 500-col chunks whose relus are
round-robined over Scalar/Vector/GpSimd; the MRU scan runs in 500-col
carry-chained chunks that chase the gate chunks, and the attention
(same exp/Z-folding algebra as before, 500-col chunks) chases the scan,
interleaved across the two batch elements to keep PE dense.
"""

import sys

sys.path.insert(0, "/opt/trn_rl_repo")

import numpy as np
import ml_dtypes

import concourse.bass as bass
import concourse.tile as tile
from concourse import bacc, mybir
from concourse.masks import make_identity

F32 = mybir.dt.float32
BF16 = mybir.dt.bfloat16
I16 = mybir.dt.int16
AX = mybir.AxisListType
OP = mybir.AluOpType
AF = mybir.ActivationFunctionType

DIM = 300
DPAD = 384
B_FULL = 16
NCORES = 8
BPC = B_FULL // NCORES  # batch per core = 2
T = 2000
TA = 2048               # article gather length (48 pad idx-0 tokens)
TQ = 30
TO = 16
RANGES = (1, 2, 4, 10, 25)
RMAX = 4608             # compacted per-core vocab rows (>= ~4.1k uniques)

DCS = 128
DC = 3

DT = BF16
NPDT = ml_dtypes.bfloat16

# z/o/B1 psum pairs (matmuls at <=512 cols, one ACT per 2-bank pair)
TP = [(0, 1024, (512, 512)), (1024, 976, (512, 464))]
# mix / scan / attention 500-col chunks
NSC = 4
SCW = 500
TSUB = 125  # attention sub-chunk (partitions of the s2 block)

# ---- packed bf16 weights: [128, 11520] ----
W_ART = 0            # 3 kc * (z|o|ce0) * 384
W_CE = 3456          # 3 kc * 4 ri * 384
W_F1 = 8064          # 3 kc * 384 (transposed)
W_F2 = 9216          # untransposed
W_F3 = 10368
WCOLS = 11520

# ---- packed f32 tensor: [128, 506] ----
F_BIAS = 0           # dc*10 + {0 bz, 1 bo, 2..6 ce_b[0..4]}
F_SCAL = 30          # 24 scalar cols (below)
F_AS1 = 54           # 6 blocks of 75 (block j = fi*3+dc)
F_AS2 = 504          # rows 0..74
F_BAS1 = 505         # rows 0..74
FCOLS = 506

SC_M1 = 0            # 15 cols: m1[k,r]/r at 5k+ri
SC_M1B = 15          # 3 cols: mr1_b
SC_M2 = 18           # 3 cols: mr2_W
SC_M2B = 21          # mr2_b
SC_AS2B = 22         # as2_b

# ---- packed i16 indices: [128, 280], idx i of a list at [i%16, base+i//16]
IX_QOPT = 0          # 256: q_b0(30+2), q_b1(30+2), opt_b0(64), opt_b1(64), pad
IX_OG0 = 16          # 64: b0 options, row layout
IX_OG1 = 20          # 64: b1 options
IX_ART0 = 24         # 2048: b0 article (+48 pad)
IX_ART1 = 152        # 2048: b1 article
IXCOLS = 280

N_WARM = 36          # dummy matmuls to trip the HAM clock gate at t=0


def _build_program():
    nc = bacc.Bacc("TRN2", target_bir_lowering=False, debug=False,
                   num_devices=NCORES, num_swdge_queues=4)

    embc = nc.dram_tensor("embc", [RMAX, DPAD], DT, kind="ExternalInput")
    idx_pack = nc.dram_tensor("idx_pack", [128, IXCOLS], I16,
                              kind="ExternalInput")
    wpack = nc.dram_tensor("wpack", [128, WCOLS], DT, kind="ExternalInput")
    fpack = nc.dram_tensor("fpack", [128, FCOLS], F32, kind="ExternalInput")
    out = nc.dram_tensor("scores", [BPC, 4], F32, kind="ExternalOutput")

    with tile.TileContext(nc) as tc:
        from contextlib import ExitStack
        with ExitStack() as ctx:
            _emit(nc, tc, ctx, embc, idx_pack, wpack, fpack, out)

    nc.compile()
    return nc


def _emit(nc, tc, ctx, embc, idx_pack, wpack, fpack, out):
    # ---------------- pools ----------------
    consts = ctx.enter_context(tc.tile_pool(name="consts", bufs=1))
    persist = ctx.enter_context(tc.tile_pool(name="persist", bufs=1))
    p_art = ctx.enter_context(tc.tile_pool(name="p_art", bufs=8))
    p_zob = ctx.enter_context(tc.tile_pool(name="p_zob", bufs=2))
    p_xs = ctx.enter_context(tc.tile_pool(name="p_xs", bufs=2))
    p_h1 = ctx.enter_context(tc.tile_pool(name="p_h1", bufs=2))
    p_gate = ctx.enter_context(tc.tile_pool(name="p_gate", bufs=3))
    p_gz = ctx.enter_context(tc.tile_pool(name="p_gz", bufs=2))
    small = ctx.enter_context(tc.tile_pool(name="small", bufs=2))
    p_e2 = ctx.enter_context(tc.tile_pool(name="p_e2", bufs=5))
    # PSUM (8 banks): pp2 2x2-bank (zob/CE-r2/pb) + mix 2x1 + work 2x1
    pp2 = ctx.enter_context(tc.tile_pool(name="pp2", bufs=2, space="PSUM"))
    pp_m = ctx.enter_context(tc.tile_pool(name="pp_m", bufs=2, space="PSUM"))
    pp_w = ctx.enter_context(tc.tile_pool(name="pp_w", bufs=2, space="PSUM"))

    # ---------------- HAM warm-up: dummy matmuls on zeroed tiles ----------
    wz_l = consts.tile([128, 128], DT)
    wz_r = consts.tile([128, 512], DT)
    nc.vector.memset(wz_l[:], 0.0)
    nc.vector.memset(wz_r[:], 0.0)
    for _ in range(N_WARM):
        ps = pp_w.tile([128, 512], F32, tag="w")
        nc.tensor.matmul(ps[:], wz_l[:], wz_r[:], start=True, stop=True)

    # ---------------- index pack + gathers ----------------
    ixp = consts.tile([128, IXCOLS], I16)
    nc.sync.dma_start(ixp[:], idx_pack[:])

    def gather(out_ap, ixcol, n, transpose, q):
        nc.gpsimd.dma_gather(
            out_ap, embc[:], ixp[:, ixcol:ixcol + (n + 15) // 16],
            n, n, DPAD, transpose=transpose, queue_num=q)

    # queue_num must track the DMASW round-robin (emission order % 4) so
    # Tile's per-queue semaphore binding stays consistent. Article b0 goes
    # first (one chunk per queue) so z/o/B1 can start ~10us in.
    # One transpose gather is capped at 512 indices: it pushes one tx
    # descriptor per index and the SWDGE ring wedges above ~512.
    artT = [[p_art.tile([128, DC, 512], DT, tag="artT", name=f"artT{b}_{c}")
             for c in range(4)] for b in range(BPC)]
    gi = 0
    for b, base in ((0, IX_ART0), (1, IX_ART1)):
        for c in range(4):
            gather(artT[b][c][:], base + 32 * c, 512, True, gi % 4)
            gi += 1
    qoptT = persist.tile([128, DC, 256], DT, tag="qoptT")
    gather(qoptT[:], IX_QOPT, 256, True, gi % 4)
    gi += 1
    # both batches' options in one row gather: b0 rows 0-63, b1 rows 64-127
    ogt = persist.tile([128, 1, DPAD], DT, tag="ogt")
    gather(ogt[:], IX_OG0, 128, False, gi % 4)
    og = [ogt[0:64], ogt[64:128]]

    # q/opt transposed views (cols within qoptT)
    def qT(b):           # [128, DC, 30]
        return qoptT[:, :, 32 * b:32 * b + TQ]

    def oT(b):           # [128, DC, 64] = (o w)
        return qoptT[:, :, 64 + 64 * b:128 + 64 * b]

    # ---------------- weights ----------------
    wp = consts.tile([128, WCOLS], DT)
    nc.sync.dma_start(wp[:], wpack[:])
    fp = consts.tile([128, FCOLS], F32)
    nc.sync.dma_start(fp[:], fpack[:])

    def w_art_v(kc, s, dc):
        o = W_ART + kc * 1152 + s * DPAD + dc * DCS
        return wp[:, o:o + DCS]

    def w_ce_v(kc, ri, dc):
        o = W_CE + kc * 1536 + ri * DPAD + dc * DCS
        return wp[:, o:o + DCS]

    def w_f_v(base, kc):
        return wp[:, base + kc * DPAD:base + (kc + 1) * DPAD]

    def bias(dc, col):
        return fp[:, dc * 10 + col:dc * 10 + col + 1]

    def sc(col):
        return fp[:, F_SCAL + col:F_SCAL + col + 1]

    ident = consts.tile([128, 128], DT)
    make_identity(nc, ident[:])

    # scaled identities for the PE-side gate mix
    mI = consts.tile([128, 18, 128], DT)
    for j in range(18):
        scol = (SC_M1 + j) if j < 15 else (SC_M2 + j - 15)
        nc.vector.tensor_scalar_mul(mI[:, j, :], ident[:], sc(scol))

    # ---------------- attention prep (needs only qoptT) ----------------
    k1T = [persist.tile([128, DC, TQ], DT, tag=f"k1T{b}", name=f"k1T{b}")
           for b in range(BPC)]
    qk_sb = [persist.tile([TQ, 132], DT, tag=f"qk{b}", name=f"qk{b}")
             for b in range(BPC)]
    for b in range(BPC):
        for dc in range(DC):
            ps = pp_w.tile([128, 512], F32, tag="w")
            for kc in range(DC):
                nc.tensor.matmul(ps[:, :TQ],
                                 w_f_v(W_F1, kc)[:, dc * DCS:(dc + 1) * DCS],
                                 qT(b)[:, kc, :], start=(kc == 0),
                                 stop=(kc == DC - 1))
            nc.scalar.copy(k1T[b][:, dc, :], ps[:, :TQ])

        aTs = []
        for fi, base in enumerate((W_F2, W_F3)):
            a_ps = pp_w.tile([TQ, DPAD], F32, tag="w")
            for kc in range(DC):
                nc.tensor.matmul(a_ps[:], qT(b)[:, kc, :], w_f_v(base, kc),
                                 start=(kc == 0), stop=(kc == DC - 1))
            a_sb = small.tile([TQ, DPAD], DT, tag="a_sb")
            nc.vector.tensor_copy(a_sb[:], a_ps[:])
            aT = persist.tile([128, DC, TQ], DT, tag=f"aT{fi}_{b}", name=f"aT{fi}_{b}")
            for dc in range(DC):
                tp = pp_w.tile([128, 512], DT, tag="w")
                nc.tensor.transpose(tp[:, :TQ],
                                    a_sb[:, dc * DCS:(dc + 1) * DCS],
                                    ident[:TQ, :TQ])
                nc.vector.tensor_copy(aT[:, dc, :], tp[:, :TQ])
            aTs.append(aT)

        qk_ps = pp_w.tile([TQ, 512], F32, tag="w")
        for fi in range(2):
            for kc in range(DC):
                nc.tensor.matmul(qk_ps[:, 64 * fi:64 * fi + 64],
                                 aTs[fi][:, kc, :], oT(b)[:, kc, :],
                                 start=(kc == 0), stop=(kc == DC - 1))
        nc.vector.tensor_copy(qk_sb[b][:, 0:128], qk_ps[:, :128])
        nc.vector.memset(qk_sb[b][:, 128:132], 1.0)

    # ---------------- group sums (xs_r in [d, g]) ----------------
    # xs2 per 512-tile (pairs are 2-aligned); xs4/xs10 from xs2; xs25 full
    # groups per tile + 3 straddle groups patched from xs2 plus one article
    # column (25g odd/even cases worked out per straddle).
    TW = [512, 512, 512, 464]
    xs = [None] * BPC
    for b in range(BPC):
        a = artT[b]
        xs2 = p_xs.tile([128, DC, T // 2], DT, tag="xs2", name=f"xs2_{b}")
        xs4 = p_xs.tile([128, DC, T // 4], DT, tag="xs4", name=f"xs4_{b}")
        xs10 = p_xs.tile([128, DC, T // 10], DT, tag="xs10", name=f"xs10_{b}")
        xs25 = p_xs.tile([128, DC, T // 25], DT, tag="xs25", name=f"xs25_{b}")
        with nc.allow_low_precision(reason="bf16 group sums"):
            for dc in range(DC):
                for c in range(4):
                    w = TW[c]
                    nc.vector.tensor_add(
                        xs2[:, dc, 256 * c:256 * c + w // 2],
                        a[c][:, dc, 0:w:2], a[c][:, dc, 1:w:2])
                    t0 = 512 * c
                    gs, ge = -(-t0 // 25), (t0 + w) // 25
                    nc.vector.tensor_reduce(
                        xs25[:, dc, gs:ge],
                        a[c][:, dc, 25 * gs - t0:25 * ge - t0].rearrange(
                            "p (g r) -> p g r", r=25),
                        AX.X, OP.add)
                for r0 in range(0, T, 500):
                    h0, h1r = r0 // 2, (r0 + 500) // 2
                    nc.gpsimd.tensor_add(xs4[:, dc, r0 // 4:(r0 + 500) // 4],
                                         xs2[:, dc, h0:h1r:2],
                                         xs2[:, dc, h0 + 1:h1r:2])
                    nc.vector.tensor_reduce(
                        xs10[:, dc, r0 // 10:(r0 + 500) // 10],
                        xs2[:, dc, h0:h1r].rearrange("p (g r) -> p g r", r=5),
                        AX.X, OP.add)
                # straddle groups: (g, xs2 col range, art tile, art col)
                for g, x0, ac, acol in ((20, 250, 1, 12), (40, 500, 2, 0),
                                        (61, 763, 2, 501)):
                    tmp = small.tile([128, 1], DT, tag="s25", name="s25")
                    nc.vector.tensor_reduce(
                        tmp[:, :],
                        xs2[:, dc, x0:x0 + 12].rearrange(
                            "p (g r) -> p g r", r=12),
                        AX.X, OP.add)
                    nc.vector.tensor_add(xs25[:, dc, g:g + 1], tmp[:],
                                         a[ac][:, dc, acol:acol + 1])
        xs[b] = dict(xs2=xs2, xs4=xs4, xs10=xs10, xs25=xs25)

    # ---------------- z / o / B1 ----------------
    zob = [None] * BPC
    for b in range(BPC):
        a = artT[b]
        z_sb = p_zob.tile([128, DC, T], DT, tag="z", name=f"z{b}")
        o_sb = p_zob.tile([128, DC, T], DT, tag="o", name=f"o{b}")
        b1_sb = p_zob.tile([128, DC, T], DT, tag="b1", name=f"b1_{b}")
        for dst, func, bcol, s in ((b1_sb, AF.Relu, 2, 2),
                                   (z_sb, AF.Tanh, 0, 0),
                                   (o_sb, AF.Tanh, 1, 1)):
            for dc in range(DC):
                for t0, tiles in ((0, (0, 1)), (1024, (2, 3))):
                    ps = pp2.tile([128, 1024], F32, tag="zo")
                    c0 = 0
                    for c in tiles:
                        w = TW[c]
                        for kc in range(DC):
                            nc.tensor.matmul(
                                ps[:, c0:c0 + w], w_art_v(kc, s, dc),
                                a[c][:, kc, 0:w],
                                start=(kc == 0), stop=(kc == DC - 1))
                        c0 += w
                    if func == AF.Relu:
                        # relu(x + b) on DVE frees the ACT engine
                        # (gpsimd cannot read PSUM)
                        nc.vector.tensor_scalar(dst[:, dc, t0:t0 + c0],
                                                ps[:, :c0], bias(dc, bcol),
                                                0.0, op0=OP.add, op1=OP.max)
                    else:
                        nc.scalar.activation(dst[:, dc, t0:t0 + c0],
                                             ps[:, :c0], func,
                                             bias=bias(dc, bcol))
        zob[b] = dict(z=z_sb, o=o_sb, b1=b1_sb)

    # ---------------- CE r>=2 (relu on DVE) ----------------
    bls = [None] * BPC
    for b in range(BPC):
        x = xs[b]
        bl = {}
        for ri, (xsr, r) in enumerate(((x["xs2"], 2), (x["xs4"], 4),
                                       (x["xs10"], 10), (x["xs25"], 25))):
            g_r = T // r
            bl[r] = p_xs.tile([128, DC, g_r], DT, tag=f"bl{r}",
                              name=f"bl{r}_{b}")
            for dc in range(DC):
                if g_r > 512:
                    ps = pp2.tile([128, 1024], F32, tag="zo")
                    for half, (g0, gn) in enumerate(((0, 512),
                                                     (512, g_r - 512))):
                        for kc in range(DC):
                            nc.tensor.matmul(
                                ps[:, half * 512:half * 512 + gn],
                                w_ce_v(kc, ri, dc), xsr[:, kc, g0:g0 + gn],
                                start=(kc == 0), stop=(kc == DC - 1))
                    nc.scalar.activation(bl[r][:, dc, :], ps[:, :g_r],
                                         AF.Relu, bias=bias(dc, 3 + ri))
                else:
                    ps = pp_w.tile([128, 512], F32, tag="w")
                    for kc in range(DC):
                        nc.tensor.matmul(ps[:, :g_r], w_ce_v(kc, ri, dc),
                                         xsr[:, kc, :], start=(kc == 0),
                                         stop=(kc == DC - 1))
                    nc.scalar.activation(bl[r][:, dc, :], ps[:, :g_r],
                                         AF.Relu, bias=bias(dc, 3 + ri))
        bls[b] = bl

    # ---------------- mix + scan + attention, chunk-pipelined -------------
    # engine round-robin for the mix relus
    _rr = [0]

    def mix_relu(dst, src, bias_ap):
        # gpsimd cannot read PSUM -> alternate ACT (2x) / DVE (1x)
        e = _rr[0] % 3
        _rr[0] += 1
        if e < 2:
            nc.scalar.activation(dst, src, AF.Relu, bias=bias_ap)
        else:
            nc.vector.tensor_scalar(dst, src, bias_ap, 0.0,
                                    op0=OP.add, op1=OP.max)

    gates = [[None] * NSC for _ in range(BPC)]   # gate chunk tiles

    def ev_chunk(b, ri, dc, t0, tn):
        r = RANGES[ri]
        if r == 1:
            return zob[b]["b1"][:, dc, t0:t0 + tn]
        return bls[b][r][:, dc, t0 // r:(t0 + tn) // r, None] \
            .to_broadcast([128, tn // r, r])

    # mix emitted per (b, chunk): for each dc: h1 k=0..2 then gate
    def emit_mix(b, sci):
        t0 = sci * SCW
        gate = p_gate.tile([128, DC, SCW], DT, tag="gate",
                           name=f"gate{b}_{sci}")
        for dc in range(DC):
            h1c = []
            for k in range(3):
                ps = pp_m.tile([128, 512], F32, tag="m")
                for ri in range(5):
                    nc.tensor.matmul(ps[:, :SCW], mI[:, 5 * k + ri, :],
                                     ev_chunk(b, ri, dc, t0, SCW),
                                     start=(ri == 0), stop=(ri == 4))
                h1 = p_h1.tile([128, SCW], DT, tag=f"h1_{k}", name=f"h1_{k}")
                mix_relu(h1[:], ps[:, :SCW], sc(SC_M1B + k))
                h1c.append(h1)
            ps = pp_m.tile([128, 512], F32, tag="m")
            for k in range(3):
                nc.tensor.matmul(ps[:, :SCW], mI[:, 15 + k, :], h1c[k][:],
                                 start=(k == 0), stop=(k == 2))
            mix_relu(gate[:, dc, :], ps[:, :SCW], sc(SC_M2B))
        gates[b][sci] = gate

    # MRU prep + scan + encode for one 500-chunk; engines alternate by dc.
    # The scan result c_t is written back into the z tile (z is dead once
    # (1-g)z is computed), so carry-in for chunk sci is z[:, dc, t0-1].
    def emit_scan(b, sci):
        t0 = sci * SCW
        gate = gates[b][sci]
        z_sb = zob[b]["z"]
        o_sb = zob[b]["o"]
        zz = p_gz.tile([128, DC, SCW], DT, tag="zz", name=f"zz{b}_{sci}")
        for dc in range(DC):
            # TensorTensor muls on gpsimd (SBUF-only engine); the scan
            # itself is a TensorScalarPtr op that only DVE supports.
            zv = z_sb[:, dc, t0:t0 + SCW]
            nc.vector.tensor_mul(zz[:, dc, :], gate[:, dc, :], zv)
            nc.vector.tensor_sub(zz[:, dc, :], zv, zz[:, dc, :])
            init = 0.0 if sci == 0 else z_sb[:, dc, t0 - 1:t0]
            nc.vector.tensor_tensor_scan(zv, gate[:, dc, :], zz[:, dc, :],
                                         init, op0=OP.mult, op1=OP.add)
            # enc chunk: o *= c  (gpsimd: SBUF-only TT, keeps DVE free)
            nc.gpsimd.tensor_mul(o_sb[:, dc, t0:t0 + SCW],
                                 o_sb[:, dc, t0:t0 + SCW], zv)

    # attention stream for one 500-chunk
    pbs = [persist.tile([128, 8], F32, tag=f"pb{b}", name=f"pb{b}")
           for b in range(BPC)]

    # attention is emitted in two PE batches per chunk so the exp/reduce/
    # reciprocal chain never blocks the in-order PE queue: (a) s1 + the 4
    # u2 matmuls; then, after a mix chunk's worth of PE work, (b) the 4 pb
    # matmuls (each 1-shot into its own psum, accumulated on DVE).
    attst = {}

    def emit_attn_a(b, sci):
        t0 = sci * SCW
        encT = zob[b]["o"]
        s1 = pp_m.tile([TQ, 512], F32, tag="m")
        for dc in range(DC):
            nc.tensor.matmul(s1[:, :SCW], k1T[b][:, dc, :],
                             encT[:, dc, t0:t0 + SCW],
                             start=(dc == 0), stop=(dc == DC - 1))
        e1T = small.tile([TQ, SCW], DT, tag="e1T")
        nc.scalar.activation(e1T[:], s1[:, :SCW], AF.Exp)
        e2s = []
        for si in range(4):
            s0 = si * TSUB
            u2 = pp_w.tile([128, 512], F32, tag="w")
            nc.tensor.matmul(u2[:TSUB, :132], e1T[:, s0:s0 + TSUB],
                             qk_sb[b][:], start=True, stop=True)
            z1 = small.tile([128, 2], F32, tag="z1")
            nc.vector.reciprocal(z1[:TSUB, 1:2], u2[:TSUB, 128:129])
            e2 = p_e2.tile([128, 128], F32, tag="e2", name=f"e2_{si}")
            nc.scalar.activation(e2[:TSUB, :], u2[:TSUB, 0:128], AF.Exp,
                                 scale=z1[:TSUB, 1:2])
            z2 = p_e2.tile([128, 16], F32, tag="z2", name=f"z2_{si}")
            nc.vector.tensor_reduce(
                z2[:TSUB, 0:8],
                e2[:TSUB, :].rearrange("p (g w) -> p g w", w=16),
                AX.X, OP.add)
            nc.vector.reciprocal(z2[:TSUB, 8:16], z2[:TSUB, 0:8])
            e2s.append((e2, z2))
        attst[(b, sci)] = e2s

    def emit_attn_b(b, sci):
        for si, (e2, z2) in enumerate(attst.pop((b, sci))):
            pb_ps = pp_w.tile([128, 512], F32, tag="w", name="pb")
            nc.tensor.matmul(pb_ps[:, :8], e2[:TSUB, :], z2[:TSUB, 8:16],
                             start=True, stop=True)
            if sci == 0 and si == 0:
                nc.vector.tensor_copy(pbs[b][:], pb_ps[:, :8])
            else:
                nc.vector.tensor_add(pbs[b][:], pbs[b][:], pb_ps[:, :8])

    # interleaved emission: keep PE dense while scans/attention chase;
    # each attention chunk's two PE batches straddle a mix chunk
    emit_mix(0, 0)
    emit_scan(0, 0)
    emit_mix(0, 1)
    emit_scan(0, 1)
    emit_mix(0, 2)
    emit_scan(0, 2)
    emit_mix(0, 3)
    emit_scan(0, 3)
    emit_attn_a(0, 0)
    emit_mix(1, 0)
    emit_scan(1, 0)
    emit_attn_b(0, 0)
    emit_attn_a(0, 1)
    emit_mix(1, 1)
    emit_scan(1, 1)
    emit_attn_b(0, 1)
    emit_attn_a(0, 2)
    emit_mix(1, 2)
    emit_scan(1, 2)
    emit_attn_b(0, 2)
    emit_attn_a(0, 3)
    emit_mix(1, 3)
    emit_scan(1, 3)
    emit_attn_b(0, 3)
    emit_attn_a(1, 0)
    emit_attn_b(1, 0)
    emit_attn_a(1, 1)
    emit_attn_b(1, 1)
    emit_attn_a(1, 2)
    emit_attn_b(1, 2)
    emit_attn_a(1, 3)
    emit_attn_b(1, 3)

    # ---------------- answer vectors + final MLP ----------------
    ans_sb = persist.tile([128, 6, 8], F32, tag="ans_sb")
    for b in range(BPC):
        pb_sb = persist.tile([128, 8], DT, tag=f"pbs{b}")
        nc.vector.tensor_copy(pb_sb[:], pbs[b][:])
        pblk_t = persist.tile([128, 8], DT, tag=f"pblk{b}")
        nc.vector.memset(pblk_t[:], 0.0)
        pblk = pblk_t[64 * b:64 * (b + 1)]
        for g in range(8):
            o = g % 4
            nc.sync.dma_start(pblk[16 * o:16 * o + 16, g:g + 1],
                              pb_sb[16 * g:16 * g + 16, g:g + 1])
        for dc in range(DC):
            ans_ps = pp_w.tile([128, 512], F32, tag="w")
            nc.tensor.matmul(ans_ps[:, :8],
                             og[b][:, 0, dc * DCS:(dc + 1) * DCS],
                             pblk[:], start=True, stop=True)
            # [:, fi*3+dc, b*4:(b+1)*4] <- ans_ps[:, fi*4:(fi+1)*4], 1/T mean
            nc.vector.tensor_scalar_mul(
                ans_sb[:, dc::3, 4 * b:4 * b + 4],
                ans_ps[:, :8].rearrange("p (f o) -> p f o", o=4), 1.0 / T)

    h_ps = pp_w.tile([75, 8], F32, tag="w")
    for j in range(6):
        nc.tensor.matmul(h_ps[:], fp[:, F_AS1 + 75 * j:F_AS1 + 75 * (j + 1)],
                         ans_sb[:, j, :], start=(j == 0), stop=(j == 5))
    h_sb = small.tile([75, 8], F32, tag="h_sb")
    nc.scalar.activation(h_sb[:], h_ps[:], AF.Relu,
                         bias=fp[0:75, F_BAS1:F_BAS1 + 1])
    s_ps = pp_m.tile([128, 512], F32, tag="m", name="s_ps")
    nc.tensor.matmul(s_ps[0:8, 0:1], h_sb[:], fp[0:75, F_AS2:F_AS2 + 1],
                     start=True, stop=True)
    s_sb = small.tile([8, 1], F32, tag="s_sb")
    nc.scalar.activation(s_sb[:], s_ps[0:8, 0:1], AF.Identity,
                         bias=fp[0:8, F_SCAL + SC_AS2B:F_SCAL + SC_AS2B + 1])
    nc.sync.dma_start(out[:].rearrange("b o -> (b o)")[:, None], s_sb[:])


# ---------------------------------------------------------------------------
# host side
# ---------------------------------------------------------------------------

_CACHE = {}


def _get_nc():
    if "nc" not in _CACHE:
        _CACHE["nc"] = _build_program()
    return _CACHE["nc"]


def _pack_weights(inputs):
    f = np.asarray
    wpack = np.zeros((128, WCOLS), np.float32)

    def pad_w(m):  # [300, 300] -> [384, 384]
        p = np.zeros((DPAD, DPAD), np.float32)
        p[:DIM, :DIM] = m
        return p

    w_art = np.zeros((DPAD, 3 * DPAD), np.float32)
    w_art[:DIM, 0 * DPAD:0 * DPAD + DIM] = f(inputs["Wz"]).T
    w_art[:DIM, 1 * DPAD:1 * DPAD + DIM] = f(inputs["Wo"]).T
    w_art[:DIM, 2 * DPAD:2 * DPAD + DIM] = f(inputs["ce_W"])[0].T
    for kc in range(DC):
        rows = slice(kc * DCS, (kc + 1) * DCS)
        wpack[:, W_ART + kc * 1152:W_ART + (kc + 1) * 1152] = w_art[rows]
        for ri in range(4):
            o = W_CE + kc * 1536 + ri * DPAD
            wpack[:, o:o + DPAD] = pad_w(f(inputs["ce_W"])[ri + 1].T)[rows]
        wpack[:, W_F1 + kc * DPAD:W_F1 + (kc + 1) * DPAD] = \
            pad_w(f(inputs["f1_W"]).T)[rows]
        # s2 = aoq @ f2W @ opt^T -> f2/f3 go in untransposed
        wpack[:, W_F2 + kc * DPAD:W_F2 + (kc + 1) * DPAD] = \
            pad_w(f(inputs["f2_W"]))[rows]
        wpack[:, W_F3 + kc * DPAD:W_F3 + (kc + 1) * DPAD] = \
            pad_w(f(inputs["f3_W"]))[rows]

    fpack = np.zeros((128, FCOLS), np.float32)
    biases = np.zeros((DPAD, 10), np.float32)
    biases[:DIM, 0] = f(inputs["bz"])
    biases[:DIM, 1] = f(inputs["bo"])
    for i in range(5):
        biases[:DIM, 2 + i] = f(inputs["ce_b"])[i]
    for kc in range(DC):
        fpack[:, F_BIAS + kc * 10:F_BIAS + (kc + 1) * 10] = \
            biases[kc * DCS:(kc + 1) * DCS]
    m1 = f(inputs["mr1_W"])
    for k in range(3):
        for ri, r in enumerate(RANGES):
            fpack[:, F_SCAL + SC_M1 + 5 * k + ri] = m1[k, ri] / r
    fpack[:, F_SCAL + SC_M1B:F_SCAL + SC_M1B + 3] = f(inputs["mr1_b"])[None, :]
    fpack[:, F_SCAL + SC_M2:F_SCAL + SC_M2 + 3] = f(inputs["mr2_W"])[0][None, :]
    fpack[:, F_SCAL + SC_M2B] = f(inputs["mr2_b"])[0]
    fpack[:, F_SCAL + SC_AS2B] = f(inputs["as2_b"])[0]
    # as1: [75, 600] -> blocks j=fi*3+dc of [128, 75]
    w_as1 = f(inputs["as1_W"])                                # [75, 600]
    for fi in range(2):
        for dc in range(DC):
            j = fi * 3 + dc
            d0 = dc * DCS
            n = min(DCS, DIM - d0) if d0 < DIM else 0
            if n > 0:
                fpack[0:n, F_AS1 + 75 * j:F_AS1 + 75 * (j + 1)] = \
                    w_as1[:, fi * DIM + d0:fi * DIM + d0 + n].T
    fpack[0:75, F_AS2] = f(inputs["as2_W"])[0]
    fpack[0:75, F_BAS1] = f(inputs["as1_b"])
    return wpack.astype(NPDT), fpack


def _wrap16(idx_list):
    """idx i -> [i % 16, i // 16] int16 column block."""
    n = len(idx_list)
    assert n % 16 == 0
    return np.asarray(idx_list, np.int16).reshape(n // 16, 16).T


def _prep_core_inputs(inputs, core):
    b0 = core * BPC
    sl = slice(b0, b0 + BPC)
    f = np.asarray
    if "prep_shared" not in _CACHE:
        wpack, fpack = _pack_weights(inputs)
        _CACHE["prep_shared"] = {
            "wpack": wpack, "fpack": fpack,
            "emb": f(inputs["emb"]).astype(np.float32),
        }
    prep = _CACHE["prep_shared"]

    art = f(inputs["article_in"])[sl].astype(np.int64)
    q = f(inputs["question_in"])[sl].astype(np.int64)
    opts = [f(inputs[f"option{o + 1}_in"])[sl].astype(np.int64)
            for o in range(4)]

    all_tok = np.concatenate([art.ravel(), q.ravel()] +
                             [o.ravel() for o in opts])
    uniq, inv = np.unique(all_tok, return_inverse=True)
    assert len(uniq) <= RMAX, f"{len(uniq)} uniques > {RMAX}"
    embc = np.zeros((RMAX, DPAD), np.float32)
    embc[:len(uniq), :DIM] = prep["emb"][uniq]

    # remapped int16 views in original shapes
    pos = 0
    art_c = inv[pos:pos + art.size].reshape(art.shape); pos += art.size
    q_c = inv[pos:pos + q.size].reshape(q.shape); pos += q.size
    opt_c = []
    for o in range(4):
        opt_c.append(inv[pos:pos + opts[o].size].reshape(opts[o].shape))
        pos += opts[o].size

    ixp = np.zeros((128, IXCOLS), np.int16)

    def put(base, idx_list):
        # idx block must be replicated across all 8 16-partition stripes:
        # each SWDGE queue's Q7 cpu pair reads its own stripe.
        blk = _wrap16(idx_list)
        for c in range(8):
            ixp[16 * c:16 * (c + 1), base:base + blk.shape[1]] = blk

    qopt = np.zeros(256, np.int64)
    qopt[0:TQ] = q_c[0]
    qopt[32:32 + TQ] = q_c[1]
    for b in range(BPC):
        for o in range(4):
            qopt[64 + 64 * b + 16 * o:64 + 64 * b + 16 * (o + 1)] = opt_c[o][b]
    put(IX_QOPT, qopt)
    ogl = np.zeros(128, np.int64)
    for b in range(BPC):
        for o in range(4):
            ogl[64 * b + 16 * o:64 * b + 16 * (o + 1)] = opt_c[o][b]
    put(IX_OG0, ogl)
    for b, base in ((0, IX_ART0), (1, IX_ART1)):
        al = np.zeros(TA, np.int64)
        al[:T] = art_c[b]
        put(base, al)

    return {
        "embc": embc.astype(NPDT),
        "idx_pack": ixp,
        "wpack": prep["wpack"],
        "fpack": prep["fpack"],
    }


def run_cores(per_core_inputs, trace=False):
    """per_core_inputs: list of 8 dicts name->np array. Returns results."""
    from concourse import bass_utils
    nc = _get_nc()
    return bass_utils.run_bass_kernel_spmd(
        nc, per_core_inputs, core_ids=list(range(NCORES)),
        trace=trace, trace_cores=[0] if trace else None)


def kernel(**inputs):
    _CACHE.pop("prep_shared", None)
    per_core = [_prep_core_inputs(inputs, c) for c in range(NCORES)]
    res = run_cores(per_core)
    out = np.concatenate([res.results[c]["scores"] for c in range(NCORES)],
                         axis=0)
    return out.astype(np.float32)


# revision 25
# speedup vs baseline: 1.2974x; 1.0236x over previous
"""BiAttentionMRU Trainium2 kernel.

Data-parallel over batch: B=16 -> 2 batch elements on each of 8 cores.
All weights replicated; the embedding is host-compacted to each core's
working set (~4k unique tokens) so the on-device gather can use the
batched SWDGE dma_gather in TRANSPOSE mode, which lands the article
directly in [d-on-partitions, token-cols] layout (no PE transposes) and
costs ~1us of gpsimd issue per 2048 tokens instead of ~17us.

Layouts: everything is [d, t] with d padded 300->384 = 3 chunks of 128
(pad rows are zeros end-to-end: emb pad cols, weight pad rows/cols and
bias pad rows are all zero, so pad lanes carry exact zeros through
z/o/gate/scan/attention).

Pipeline: a burst of zero-weight dummy matmuls at t=0 trips the PE HAM
clock gate to 2.4GHz before the real GEMMs arrive; z/o/B1 + CE stream
per batch as the gathers land; the 5->3->1 gate mix runs as
scaled-identity accumulating matmuls in 500-col chunks whose relus are
round-robined over Scalar/Vector/GpSimd; the MRU scan runs in 500-col
carry-chained chunks that chase the gate chunks, and the attention
(same exp/Z-folding algebra as before, 500-col chunks) chases the scan,
interleaved across the two batch elements to keep PE dense.
"""

import os
import sys

sys.path.insert(0, "/opt/trn_rl_repo")

# heal any degraded power/clock state left by a previous run
os.environ.setdefault("NEURON_RT_RESET_CORES", "1")

import numpy as np
import ml_dtypes

import concourse.bass as bass
import concourse.tile as tile
from concourse import bacc, mybir
from concourse.masks import make_identity

F32 = mybir.dt.float32
BF16 = mybir.dt.bfloat16
I16 = mybir.dt.int16
AX = mybir.AxisListType
OP = mybir.AluOpType
AF = mybir.ActivationFunctionType

DIM = 300
DPAD = 384
B_FULL = 16
NCORES = 8
BPC = B_FULL // NCORES  # batch per core = 2
T = 2000
TA = 2048               # article gather length (48 pad idx-0 tokens)
TQ = 30
TO = 16
RANGES = (1, 2, 4, 10, 25)
RMAX = 4608             # compacted per-core vocab rows (>= ~4.1k uniques)

DCS = 128
DC = 3

DT = BF16
NPDT = ml_dtypes.bfloat16

# z/o/B1 psum pairs (matmuls at <=512 cols, one ACT per 2-bank pair)
TP = [(0, 1024, (512, 512)), (1024, 976, (512, 464))]
# mix / scan / attention 500-col chunks
NSC = 4
SCW = 500
TSUB = 125  # attention sub-chunk (partitions of the s2 block)

# ---- packed bf16 weights: [128, 11520] ----
W_ART = 0            # 3 kc * (z|o|ce0) * 384
W_CE = 3456          # 3 kc * 4 ri * 384
W_F1 = 8064          # 3 kc * 384 (transposed)
W_F2 = 9216          # untransposed
W_F3 = 10368
WCOLS = 11520

# ---- packed f32 tensor: [128, 506] ----
F_BIAS = 0           # dc*10 + {0 bz, 1 bo, 2..6 ce_b[0..4]}
F_SCAL = 30          # 24 scalar cols (below)
F_AS1 = 54           # 6 blocks of 75 (block j = fi*3+dc)
F_AS2 = 504          # rows 0..74
F_BAS1 = 505         # rows 0..74
FCOLS = 506

SC_M1 = 0            # 15 cols: m1[k,r]/r at 5k+ri
SC_M1B = 15          # 3 cols: mr1_b
SC_M2 = 18           # 3 cols: mr2_W
SC_M2B = 21          # mr2_b
SC_AS2B = 22         # as2_b

# ---- packed i16 indices: [128, 280], idx i of a list at [i%16, base+i//16]
IX_QOPT = 0          # 256: q_b0(30+2), q_b1(30+2), opt_b0(64), opt_b1(64), pad
IX_OG0 = 16          # 64: b0 options, row layout
IX_OG1 = 20          # 64: b1 options
IX_ART0 = 24         # 2048: b0 article (+48 pad)
IX_ART1 = 152        # 2048: b1 article
IXCOLS = 280

N_WARM = 55          # dummy matmuls to bridge PE until the first gathers land


def _build_program():
    nc = bacc.Bacc("TRN2", target_bir_lowering=False, debug=False,
                   num_devices=NCORES, num_swdge_queues=4)

    embc = nc.dram_tensor("embc", [RMAX, DPAD], DT, kind="ExternalInput")
    idx_pack = nc.dram_tensor("idx_pack", [128, IXCOLS], I16,
                              kind="ExternalInput")
    wpack = nc.dram_tensor("wpack", [128, WCOLS], DT, kind="ExternalInput")
    fpack = nc.dram_tensor("fpack", [128, FCOLS], F32, kind="ExternalInput")
    out = nc.dram_tensor("scores", [BPC, 4], F32, kind="ExternalOutput")

    with tile.TileContext(nc) as tc:
        from contextlib import ExitStack
        with ExitStack() as ctx:
            _emit(nc, tc, ctx, embc, idx_pack, wpack, fpack, out)

    nc.compile()
    return nc


def _emit(nc, tc, ctx, embc, idx_pack, wpack, fpack, out):
    # ---------------- pools ----------------
    consts = ctx.enter_context(tc.tile_pool(name="consts", bufs=1))
    persist = ctx.enter_context(tc.tile_pool(name="persist", bufs=1))
    p_art = ctx.enter_context(tc.tile_pool(name="p_art", bufs=8))
    p_zob = ctx.enter_context(tc.tile_pool(name="p_zob", bufs=2))
    p_xs = ctx.enter_context(tc.tile_pool(name="p_xs", bufs=2))
    p_h1 = ctx.enter_context(tc.tile_pool(name="p_h1", bufs=2))
    p_gate = ctx.enter_context(tc.tile_pool(name="p_gate", bufs=3))
    p_gz = ctx.enter_context(tc.tile_pool(name="p_gz", bufs=2))
    small = ctx.enter_context(tc.tile_pool(name="small", bufs=2))
    p_e2 = ctx.enter_context(tc.tile_pool(name="p_e2", bufs=9))
    # PSUM (8 banks): pp2 2x2-bank (zob/CE-r2/pb) + mix 2x1 + work 2x1
    pp2 = ctx.enter_context(tc.tile_pool(name="pp2", bufs=2, space="PSUM"))
    pp_m = ctx.enter_context(tc.tile_pool(name="pp_m", bufs=2, space="PSUM"))
    pp_w = ctx.enter_context(tc.tile_pool(name="pp_w", bufs=2, space="PSUM"))

    # ---------------- HAM warm-up: dummy matmuls on zeroed tiles ----------
    wz_l = consts.tile([128, 128], DT)
    wz_r = consts.tile([128, 512], DT)
    nc.vector.memset(wz_l[:], 0.0)
    nc.vector.memset(wz_r[:], 0.0)
    for _ in range(N_WARM):
        ps = pp_w.tile([128, 512], F32, tag="w")
        nc.tensor.matmul(ps[:], wz_l[:], wz_r[:], start=True, stop=True)

    # ---------------- index pack + gathers ----------------
    ixp = consts.tile([128, IXCOLS], I16)
    nc.sync.dma_start(ixp[:], idx_pack[:])

    def gather(out_ap, ixcol, n, transpose, q):
        nc.gpsimd.dma_gather(
            out_ap, embc[:], ixp[:, ixcol:ixcol + (n + 15) // 16],
            n, n, DPAD, transpose=transpose, queue_num=q)

    # queue_num must track the DMASW round-robin (emission order % 4) so
    # Tile's per-queue semaphore binding stays consistent. Article b0 goes
    # first (one chunk per queue) so z/o/B1 can start ~10us in.
    # One transpose gather is capped at 512 indices: it pushes one tx
    # descriptor per index and the SWDGE ring wedges above ~512.
    artT = [[p_art.tile([128, DC, 512], DT, tag="artT", name=f"artT{b}_{c}")
             for c in range(4)] for b in range(BPC)]
    gi = 0
    for b, base in ((0, IX_ART0), (1, IX_ART1)):
        for c in range(4):
            gather(artT[b][c][:], base + 32 * c, 512, True, gi % 4)
            gi += 1
    qoptT = persist.tile([128, DC, 256], DT, tag="qoptT")
    gather(qoptT[:], IX_QOPT, 256, True, gi % 4)
    gi += 1
    # both batches' options in one row gather: b0 rows 0-63, b1 rows 64-127
    ogt = persist.tile([128, 1, DPAD], DT, tag="ogt")
    gather(ogt[:], IX_OG0, 128, False, gi % 4)
    og = [ogt[0:64], ogt[64:128]]

    # q/opt transposed views (cols within qoptT)
    def qT(b):           # [128, DC, 30]
        return qoptT[:, :, 32 * b:32 * b + TQ]

    def oT(b):           # [128, DC, 64] = (o w)
        return qoptT[:, :, 64 + 64 * b:128 + 64 * b]

    # ---------------- weights ----------------
    wp = consts.tile([128, WCOLS], DT)
    nc.sync.dma_start(wp[:], wpack[:])
    fp = consts.tile([128, FCOLS], F32)
    nc.sync.dma_start(fp[:], fpack[:])

    def w_art_v(kc, s, dc):
        o = W_ART + kc * 1152 + s * DPAD + dc * DCS
        return wp[:, o:o + DCS]

    def w_ce_v(kc, ri, dc):
        o = W_CE + kc * 1536 + ri * DPAD + dc * DCS
        return wp[:, o:o + DCS]

    def w_f_v(base, kc):
        return wp[:, base + kc * DPAD:base + (kc + 1) * DPAD]

    def bias(dc, col):
        return fp[:, dc * 10 + col:dc * 10 + col + 1]

    def sc(col):
        return fp[:, F_SCAL + col:F_SCAL + col + 1]

    ident = consts.tile([128, 128], DT)
    make_identity(nc, ident[:])

    # scaled identities for the PE-side gate mix
    mI = consts.tile([128, 18, 128], DT)
    for j in range(18):
        scol = (SC_M1 + j) if j < 15 else (SC_M2 + j - 15)
        nc.vector.tensor_scalar_mul(mI[:, j, :], ident[:], sc(scol))

    # ---------------- attention prep (needs only qoptT) ----------------
    k1T = [persist.tile([128, DC, TQ], DT, tag=f"k1T{b}", name=f"k1T{b}")
           for b in range(BPC)]
    qk_sb = [persist.tile([TQ, 132], DT, tag=f"qk{b}", name=f"qk{b}")
             for b in range(BPC)]
    for b in range(BPC):
        for dc in range(DC):
            ps = pp_w.tile([128, 512], F32, tag="w")
            for kc in range(DC):
                nc.tensor.matmul(ps[:, :TQ],
                                 w_f_v(W_F1, kc)[:, dc * DCS:(dc + 1) * DCS],
                                 qT(b)[:, kc, :], start=(kc == 0),
                                 stop=(kc == DC - 1))
            nc.scalar.copy(k1T[b][:, dc, :], ps[:, :TQ])

        aTs = []
        for fi, base in enumerate((W_F2, W_F3)):
            a_ps = pp_w.tile([TQ, DPAD], F32, tag="w")
            for kc in range(DC):
                nc.tensor.matmul(a_ps[:], qT(b)[:, kc, :], w_f_v(base, kc),
                                 start=(kc == 0), stop=(kc == DC - 1))
            a_sb = small.tile([TQ, DPAD], DT, tag="a_sb")
            nc.vector.tensor_copy(a_sb[:], a_ps[:])
            aT = persist.tile([128, DC, TQ], DT, tag=f"aT{fi}_{b}", name=f"aT{fi}_{b}")
            for dc in range(DC):
                tp = pp_w.tile([128, 512], DT, tag="w")
                nc.tensor.transpose(tp[:, :TQ],
                                    a_sb[:, dc * DCS:(dc + 1) * DCS],
                                    ident[:TQ, :TQ])
                nc.vector.tensor_copy(aT[:, dc, :], tp[:, :TQ])
            aTs.append(aT)

        qk_ps = pp_w.tile([TQ, 512], F32, tag="w")
        for fi in range(2):
            for kc in range(DC):
                nc.tensor.matmul(qk_ps[:, 64 * fi:64 * fi + 64],
                                 aTs[fi][:, kc, :], oT(b)[:, kc, :],
                                 start=(kc == 0), stop=(kc == DC - 1))
        nc.vector.tensor_copy(qk_sb[b][:, 0:128], qk_ps[:, :128])
        nc.vector.memset(qk_sb[b][:, 128:132], 1.0)

    # ---------------- group sums (xs_r in [d, g]) ----------------
    # xs2 per 512-tile (pairs are 2-aligned); xs4/xs10 from xs2; xs25 full
    # groups per tile + 3 straddle groups patched from xs2 plus one article
    # column (25g odd/even cases worked out per straddle).
    TW = [512, 512, 512, 464]
    xs = [None] * BPC
    for b in range(BPC):
        a = artT[b]
        xs2 = p_xs.tile([128, DC, T // 2], DT, tag="xs2", name=f"xs2_{b}")
        xs4 = p_xs.tile([128, DC, T // 4], DT, tag="xs4", name=f"xs4_{b}")
        xs10 = p_xs.tile([128, DC, T // 10], DT, tag="xs10", name=f"xs10_{b}")
        xs25 = p_xs.tile([128, DC, T // 25], DT, tag="xs25", name=f"xs25_{b}")
        with nc.allow_low_precision(reason="bf16 group sums"):
            for dc in range(DC):
                for c in range(4):
                    w = TW[c]
                    nc.vector.tensor_add(
                        xs2[:, dc, 256 * c:256 * c + w // 2],
                        a[c][:, dc, 0:w:2], a[c][:, dc, 1:w:2])
                    t0 = 512 * c
                    gs, ge = -(-t0 // 25), (t0 + w) // 25
                    nc.vector.tensor_reduce(
                        xs25[:, dc, gs:ge],
                        a[c][:, dc, 25 * gs - t0:25 * ge - t0].rearrange(
                            "p (g r) -> p g r", r=25),
                        AX.X, OP.add)
                for r0 in range(0, T, 500):
                    h0, h1r = r0 // 2, (r0 + 500) // 2
                    nc.gpsimd.tensor_add(xs4[:, dc, r0 // 4:(r0 + 500) // 4],
                                         xs2[:, dc, h0:h1r:2],
                                         xs2[:, dc, h0 + 1:h1r:2])
                    nc.vector.tensor_reduce(
                        xs10[:, dc, r0 // 10:(r0 + 500) // 10],
                        xs2[:, dc, h0:h1r].rearrange("p (g r) -> p g r", r=5),
                        AX.X, OP.add)
                # straddle groups: (g, xs2 col range, art tile, art col)
                for g, x0, ac, acol in ((20, 250, 1, 12), (40, 500, 2, 0),
                                        (61, 763, 2, 501)):
                    tmp = small.tile([128, 1], DT, tag="s25", name="s25")
                    nc.vector.tensor_reduce(
                        tmp[:, :],
                        xs2[:, dc, x0:x0 + 12].rearrange(
                            "p (g r) -> p g r", r=12),
                        AX.X, OP.add)
                    nc.vector.tensor_add(xs25[:, dc, g:g + 1], tmp[:],
                                         a[ac][:, dc, acol:acol + 1])
        xs[b] = dict(xs2=xs2, xs4=xs4, xs10=xs10, xs25=xs25)

    # ---------------- z / o / B1 ----------------
    zob = [None] * BPC
    for b in range(BPC):
        a = artT[b]
        z_sb = p_zob.tile([128, DC, T], DT, tag="z", name=f"z{b}")
        o_sb = p_zob.tile([128, DC, T], DT, tag="o", name=f"o{b}")
        b1_sb = p_zob.tile([128, DC, T], DT, tag="b1", name=f"b1_{b}")
        for dst, func, bcol, s in ((b1_sb, AF.Relu, 2, 2),
                                   (z_sb, AF.Tanh, 0, 0),
                                   (o_sb, AF.Tanh, 1, 1)):
            for dc in range(DC):
                for t0, tiles in ((0, (0, 1)), (1024, (2, 3))):
                    ps = pp2.tile([128, 1024], F32, tag="zo")
                    c0 = 0
                    for c in tiles:
                        w = TW[c]
                        for kc in range(DC):
                            nc.tensor.matmul(
                                ps[:, c0:c0 + w], w_art_v(kc, s, dc),
                                a[c][:, kc, 0:w],
                                start=(kc == 0), stop=(kc == DC - 1))
                        c0 += w
                    if func == AF.Relu:
                        # relu(x + b) on DVE frees the ACT engine
                        # (gpsimd cannot read PSUM)
                        nc.vector.tensor_scalar(dst[:, dc, t0:t0 + c0],
                                                ps[:, :c0], bias(dc, bcol),
                                                0.0, op0=OP.add, op1=OP.max)
                    else:
                        nc.scalar.activation(dst[:, dc, t0:t0 + c0],
                                             ps[:, :c0], func,
                                             bias=bias(dc, bcol))
        zob[b] = dict(z=z_sb, o=o_sb, b1=b1_sb)

    # ---------------- CE r>=2 (relu on DVE) ----------------
    bls = [None] * BPC
    for b in range(BPC):
        x = xs[b]
        bl = {}
        for ri, (xsr, r) in enumerate(((x["xs2"], 2), (x["xs4"], 4),
                                       (x["xs10"], 10), (x["xs25"], 25))):
            g_r = T // r
            bl[r] = p_xs.tile([128, DC, g_r], DT, tag=f"bl{r}",
                              name=f"bl{r}_{b}")
            for dc in range(DC):
                if g_r > 512:
                    ps = pp2.tile([128, 1024], F32, tag="zo")
                    for half, (g0, gn) in enumerate(((0, 512),
                                                     (512, g_r - 512))):
                        for kc in range(DC):
                            nc.tensor.matmul(
                                ps[:, half * 512:half * 512 + gn],
                                w_ce_v(kc, ri, dc), xsr[:, kc, g0:g0 + gn],
                                start=(kc == 0), stop=(kc == DC - 1))
                    nc.scalar.activation(bl[r][:, dc, :], ps[:, :g_r],
                                         AF.Relu, bias=bias(dc, 3 + ri))
                else:
                    ps = pp_w.tile([128, 512], F32, tag="w")
                    for kc in range(DC):
                        nc.tensor.matmul(ps[:, :g_r], w_ce_v(kc, ri, dc),
                                         xsr[:, kc, :], start=(kc == 0),
                                         stop=(kc == DC - 1))
                    nc.scalar.activation(bl[r][:, dc, :], ps[:, :g_r],
                                         AF.Relu, bias=bias(dc, 3 + ri))
        bls[b] = bl

    # ---------------- mix + scan + attention, chunk-pipelined -------------
    # engine round-robin for the mix relus
    _rr = [0]

    def mix_relu(dst, src, bias_ap):
        # gpsimd cannot read PSUM -> alternate ACT (2x) / DVE (1x)
        e = _rr[0] % 3
        _rr[0] += 1
        if e < 2:
            nc.scalar.activation(dst, src, AF.Relu, bias=bias_ap)
        else:
            nc.vector.tensor_scalar(dst, src, bias_ap, 0.0,
                                    op0=OP.add, op1=OP.max)

    gates = [[None] * NSC for _ in range(BPC)]   # gate chunk tiles

    def ev_chunk(b, ri, dc, t0, tn):
        r = RANGES[ri]
        if r == 1:
            return zob[b]["b1"][:, dc, t0:t0 + tn]
        return bls[b][r][:, dc, t0 // r:(t0 + tn) // r, None] \
            .to_broadcast([128, tn // r, r])

    # mix emitted per (b, chunk): for each dc: h1 k=0..2 then gate
    def emit_mix(b, sci):
        t0 = sci * SCW
        gate = p_gate.tile([128, DC, SCW], DT, tag="gate",
                           name=f"gate{b}_{sci}")
        for dc in range(DC):
            h1c = []
            for k in range(3):
                ps = pp_m.tile([128, 512], F32, tag="m")
                for ri in range(5):
                    nc.tensor.matmul(ps[:, :SCW], mI[:, 5 * k + ri, :],
                                     ev_chunk(b, ri, dc, t0, SCW),
                                     start=(ri == 0), stop=(ri == 4))
                h1 = p_h1.tile([128, SCW], DT, tag=f"h1_{k}", name=f"h1_{k}")
                mix_relu(h1[:], ps[:, :SCW], sc(SC_M1B + k))
                h1c.append(h1)
            ps = pp_m.tile([128, 512], F32, tag="m")
            for k in range(3):
                nc.tensor.matmul(ps[:, :SCW], mI[:, 15 + k, :], h1c[k][:],
                                 start=(k == 0), stop=(k == 2))
            mix_relu(gate[:, dc, :], ps[:, :SCW], sc(SC_M2B))
        gates[b][sci] = gate

    # MRU prep + scan + encode for one 500-chunk; engines alternate by dc.
    # The scan result c_t is written back into the z tile (z is dead once
    # (1-g)z is computed), so carry-in for chunk sci is z[:, dc, t0-1].
    def emit_scan(b, sci):
        t0 = sci * SCW
        gate = gates[b][sci]
        z_sb = zob[b]["z"]
        o_sb = zob[b]["o"]
        zz = p_gz.tile([128, DC, SCW], DT, tag="zz", name=f"zz{b}_{sci}")
        for dc in range(DC):
            # TensorTensor muls on gpsimd (SBUF-only engine); the scan
            # itself is a TensorScalarPtr op that only DVE supports.
            zv = z_sb[:, dc, t0:t0 + SCW]
            nc.vector.tensor_mul(zz[:, dc, :], gate[:, dc, :], zv)
            nc.vector.tensor_sub(zz[:, dc, :], zv, zz[:, dc, :])
            init = 0.0 if sci == 0 else z_sb[:, dc, t0 - 1:t0]
            nc.vector.tensor_tensor_scan(zv, gate[:, dc, :], zz[:, dc, :],
                                         init, op0=OP.mult, op1=OP.add)
            # enc chunk: o *= c  (gpsimd: SBUF-only TT, keeps DVE free)
            nc.gpsimd.tensor_mul(o_sb[:, dc, t0:t0 + SCW],
                                 o_sb[:, dc, t0:t0 + SCW], zv)

    # attention stream for one 500-chunk
    pbs = [persist.tile([128, 8], F32, tag=f"pb{b}", name=f"pb{b}")
           for b in range(BPC)]

    # attention is emitted in two PE batches per chunk so the exp/reduce/
    # reciprocal chain never blocks the in-order PE queue: (a) s1 + the 4
    # u2 matmuls; then, after a mix chunk's worth of PE work, (b) the 4 pb
    # matmuls (each 1-shot into its own psum, accumulated on DVE).
    attst = {}

    def emit_attn_a(b, sci):
        t0 = sci * SCW
        encT = zob[b]["o"]
        s1 = pp_m.tile([TQ, 512], F32, tag="m")
        for dc in range(DC):
            nc.tensor.matmul(s1[:, :SCW], k1T[b][:, dc, :],
                             encT[:, dc, t0:t0 + SCW],
                             start=(dc == 0), stop=(dc == DC - 1))
        e1T = small.tile([TQ, SCW], DT, tag="e1T")
        nc.scalar.activation(e1T[:], s1[:, :SCW], AF.Exp)
        e2s = []
        for si in range(4):
            s0 = si * TSUB
            u2 = pp_w.tile([128, 512], F32, tag="w")
            nc.tensor.matmul(u2[:TSUB, :132], e1T[:, s0:s0 + TSUB],
                             qk_sb[b][:], start=True, stop=True)
            z1 = small.tile([128, 2], F32, tag="z1")
            nc.vector.reciprocal(z1[:TSUB, 1:2], u2[:TSUB, 128:129])
            e2 = p_e2.tile([128, 128], F32, tag="e2", name=f"e2_{si}")
            nc.scalar.activation(e2[:TSUB, :], u2[:TSUB, 0:128], AF.Exp,
                                 scale=z1[:TSUB, 1:2])
            z2 = p_e2.tile([128, 16], F32, tag="z2", name=f"z2_{si}")
            nc.vector.tensor_reduce(
                z2[:TSUB, 0:8],
                e2[:TSUB, :].rearrange("p (g w) -> p g w", w=16),
                AX.X, OP.add)
            nc.vector.reciprocal(z2[:TSUB, 8:16], z2[:TSUB, 0:8])
            e2s.append((e2, z2))
        attst[(b, sci)] = e2s

    def emit_attn_b(b, sci):
        for si, (e2, z2) in enumerate(attst.pop((b, sci))):
            pb_ps = pp_w.tile([128, 512], F32, tag="w", name="pb")
            nc.tensor.matmul(pb_ps[:, :8], e2[:TSUB, :], z2[:TSUB, 8:16],
                             start=True, stop=True)
            if sci == 0 and si == 0:
                nc.vector.tensor_copy(pbs[b][:], pb_ps[:, :8])
            else:
                nc.vector.tensor_add(pbs[b][:], pbs[b][:], pb_ps[:, :8])

    # interleaved emission: keep PE dense while scans/attention chase;
    # each attention chunk's two PE batches straddle a mix chunk
    emit_mix(0, 0)
    emit_scan(0, 0)
    emit_mix(0, 1)
    emit_scan(0, 1)
    emit_mix(0, 2)
    emit_scan(0, 2)
    emit_mix(0, 3)
    emit_scan(0, 3)
    emit_mix(1, 0)
    emit_scan(1, 0)
    emit_attn_a(0, 0)
    emit_attn_a(0, 1)
    emit_mix(1, 1)
    emit_scan(1, 1)
    emit_attn_b(0, 0)
    emit_attn_b(0, 1)
    emit_attn_a(0, 2)
    emit_attn_a(0, 3)
    emit_mix(1, 2)
    emit_scan(1, 2)
    emit_attn_b(0, 2)
    emit_attn_b(0, 3)
    emit_attn_a(1, 0)
    emit_attn_a(1, 1)
    emit_mix(1, 3)
    emit_scan(1, 3)
    emit_attn_b(1, 0)
    emit_attn_b(1, 1)
    emit_attn_a(1, 2)
    emit_attn_a(1, 3)
    emit_attn_b(1, 2)
    emit_attn_b(1, 3)

    # ---------------- answer vectors + final MLP ----------------
    ans_sb = persist.tile([128, 6, 8], F32, tag="ans_sb")
    for b in range(BPC):
        pb_sb = persist.tile([128, 8], DT, tag=f"pbs{b}")
        nc.vector.tensor_copy(pb_sb[:], pbs[b][:])
        pblk_t = persist.tile([128, 8], DT, tag=f"pblk{b}")
        nc.vector.memset(pblk_t[:], 0.0)
        pblk = pblk_t[64 * b:64 * (b + 1)]
        for g in range(8):
            o = g % 4
            nc.sync.dma_start(pblk[16 * o:16 * o + 16, g:g + 1],
                              pb_sb[16 * g:16 * g + 16, g:g + 1])
        for dc in range(DC):
            ans_ps = pp_w.tile([128, 512], F32, tag="w")
            nc.tensor.matmul(ans_ps[:, :8],
                             og[b][:, 0, dc * DCS:(dc + 1) * DCS],
                             pblk[:], start=True, stop=True)
            # [:, fi*3+dc, b*4:(b+1)*4] <- ans_ps[:, fi*4:(fi+1)*4], 1/T mean
            nc.vector.tensor_scalar_mul(
                ans_sb[:, dc::3, 4 * b:4 * b + 4],
                ans_ps[:, :8].rearrange("p (f o) -> p f o", o=4), 1.0 / T)

    h_ps = pp_w.tile([75, 8], F32, tag="w")
    for j in range(6):
        nc.tensor.matmul(h_ps[:], fp[:, F_AS1 + 75 * j:F_AS1 + 75 * (j + 1)],
                         ans_sb[:, j, :], start=(j == 0), stop=(j == 5))
    h_sb = small.tile([75, 8], F32, tag="h_sb")
    nc.scalar.activation(h_sb[:], h_ps[:], AF.Relu,
                         bias=fp[0:75, F_BAS1:F_BAS1 + 1])
    s_ps = pp_m.tile([128, 512], F32, tag="m", name="s_ps")
    nc.tensor.matmul(s_ps[0:8, 0:1], h_sb[:], fp[0:75, F_AS2:F_AS2 + 1],
                     start=True, stop=True)
    s_sb = small.tile([8, 1], F32, tag="s_sb")
    nc.scalar.activation(s_sb[:], s_ps[0:8, 0:1], AF.Identity,
                         bias=fp[0:8, F_SCAL + SC_AS2B:F_SCAL + SC_AS2B + 1])
    nc.sync.dma_start(out[:].rearrange("b o -> (b o)")[:, None], s_sb[:])


# ---------------------------------------------------------------------------
# host side
# ---------------------------------------------------------------------------

_CACHE = {}


def _get_nc():
    if "nc" not in _CACHE:
        _CACHE["nc"] = _build_program()
    return _CACHE["nc"]


def _pack_weights(inputs):
    f = np.asarray
    wpack = np.zeros((128, WCOLS), np.float32)

    def pad_w(m):  # [300, 300] -> [384, 384]
        p = np.zeros((DPAD, DPAD), np.float32)
        p[:DIM, :DIM] = m
        return p

    w_art = np.zeros((DPAD, 3 * DPAD), np.float32)
    w_art[:DIM, 0 * DPAD:0 * DPAD + DIM] = f(inputs["Wz"]).T
    w_art[:DIM, 1 * DPAD:1 * DPAD + DIM] = f(inputs["Wo"]).T
    w_art[:DIM, 2 * DPAD:2 * DPAD + DIM] = f(inputs["ce_W"])[0].T
    for kc in range(DC):
        rows = slice(kc * DCS, (kc + 1) * DCS)
        wpack[:, W_ART + kc * 1152:W_ART + (kc + 1) * 1152] = w_art[rows]
        for ri in range(4):
            o = W_CE + kc * 1536 + ri * DPAD
            wpack[:, o:o + DPAD] = pad_w(f(inputs["ce_W"])[ri + 1].T)[rows]
        wpack[:, W_F1 + kc * DPAD:W_F1 + (kc + 1) * DPAD] = \
            pad_w(f(inputs["f1_W"]).T)[rows]
        # s2 = aoq @ f2W @ opt^T -> f2/f3 go in untransposed
        wpack[:, W_F2 + kc * DPAD:W_F2 + (kc + 1) * DPAD] = \
            pad_w(f(inputs["f2_W"]))[rows]
        wpack[:, W_F3 + kc * DPAD:W_F3 + (kc + 1) * DPAD] = \
            pad_w(f(inputs["f3_W"]))[rows]

    fpack = np.zeros((128, FCOLS), np.float32)
    biases = np.zeros((DPAD, 10), np.float32)
    biases[:DIM, 0] = f(inputs["bz"])
    biases[:DIM, 1] = f(inputs["bo"])
    for i in range(5):
        biases[:DIM, 2 + i] = f(inputs["ce_b"])[i]
    for kc in range(DC):
        fpack[:, F_BIAS + kc * 10:F_BIAS + (kc + 1) * 10] = \
            biases[kc * DCS:(kc + 1) * DCS]
    m1 = f(inputs["mr1_W"])
    for k in range(3):
        for ri, r in enumerate(RANGES):
            fpack[:, F_SCAL + SC_M1 + 5 * k + ri] = m1[k, ri] / r
    fpack[:, F_SCAL + SC_M1B:F_SCAL + SC_M1B + 3] = f(inputs["mr1_b"])[None, :]
    fpack[:, F_SCAL + SC_M2:F_SCAL + SC_M2 + 3] = f(inputs["mr2_W"])[0][None, :]
    fpack[:, F_SCAL + SC_M2B] = f(inputs["mr2_b"])[0]
    fpack[:, F_SCAL + SC_AS2B] = f(inputs["as2_b"])[0]
    # as1: [75, 600] -> blocks j=fi*3+dc of [128, 75]
    w_as1 = f(inputs["as1_W"])                                # [75, 600]
    for fi in range(2):
        for dc in range(DC):
            j = fi * 3 + dc
            d0 = dc * DCS
            n = min(DCS, DIM - d0) if d0 < DIM else 0
            if n > 0:
                fpack[0:n, F_AS1 + 75 * j:F_AS1 + 75 * (j + 1)] = \
                    w_as1[:, fi * DIM + d0:fi * DIM + d0 + n].T
    fpack[0:75, F_AS2] = f(inputs["as2_W"])[0]
    fpack[0:75, F_BAS1] = f(inputs["as1_b"])
    return wpack.astype(NPDT), fpack


def _wrap16(idx_list):
    """idx i -> [i % 16, i // 16] int16 column block."""
    n = len(idx_list)
    assert n % 16 == 0
    return np.asarray(idx_list, np.int16).reshape(n // 16, 16).T


def _prep_core_inputs(inputs, core):
    b0 = core * BPC
    sl = slice(b0, b0 + BPC)
    f = np.asarray
    if "prep_shared" not in _CACHE:
        wpack, fpack = _pack_weights(inputs)
        _CACHE["prep_shared"] = {
            "wpack": wpack, "fpack": fpack,
            "emb": f(inputs["emb"]).astype(np.float32),
        }
    prep = _CACHE["prep_shared"]

    art = f(inputs["article_in"])[sl].astype(np.int64)
    q = f(inputs["question_in"])[sl].astype(np.int64)
    opts = [f(inputs[f"option{o + 1}_in"])[sl].astype(np.int64)
            for o in range(4)]

    all_tok = np.concatenate([art.ravel(), q.ravel()] +
                             [o.ravel() for o in opts])
    uniq, inv = np.unique(all_tok, return_inverse=True)
    assert len(uniq) <= RMAX, f"{len(uniq)} uniques > {RMAX}"
    embc = np.zeros((RMAX, DPAD), np.float32)
    embc[:len(uniq), :DIM] = prep["emb"][uniq]

    # remapped int16 views in original shapes
    pos = 0
    art_c = inv[pos:pos + art.size].reshape(art.shape); pos += art.size
    q_c = inv[pos:pos + q.size].reshape(q.shape); pos += q.size
    opt_c = []
    for o in range(4):
        opt_c.append(inv[pos:pos + opts[o].size].reshape(opts[o].shape))
        pos += opts[o].size

    ixp = np.zeros((128, IXCOLS), np.int16)

    def put(base, idx_list):
        # idx block must be replicated across all 8 16-partition stripes:
        # each SWDGE queue's Q7 cpu pair reads its own stripe.
        blk = _wrap16(idx_list)
        for c in range(8):
            ixp[16 * c:16 * (c + 1), base:base + blk.shape[1]] = blk

    qopt = np.zeros(256, np.int64)
    qopt[0:TQ] = q_c[0]
    qopt[32:32 + TQ] = q_c[1]
    for b in range(BPC):
        for o in range(4):
            qopt[64 + 64 * b + 16 * o:64 + 64 * b + 16 * (o + 1)] = opt_c[o][b]
    put(IX_QOPT, qopt)
    ogl = np.zeros(128, np.int64)
    for b in range(BPC):
        for o in range(4):
            ogl[64 * b + 16 * o:64 * b + 16 * (o + 1)] = opt_c[o][b]
    put(IX_OG0, ogl)
    for b, base in ((0, IX_ART0), (1, IX_ART1)):
        al = np.zeros(TA, np.int64)
        al[:T] = art_c[b]
        put(base, al)

    return {
        "embc": embc.astype(NPDT),
        "idx_pack": ixp,
        "wpack": prep["wpack"],
        "fpack": prep["fpack"],
    }


def run_cores(per_core_inputs, trace=False):
    """per_core_inputs: list of 8 dicts name->np array. Returns results."""
    from concourse import bass_utils
    nc = _get_nc()
    return bass_utils.run_bass_kernel_spmd(
        nc, per_core_inputs, core_ids=list(range(NCORES)),
        trace=trace, trace_cores=[0] if trace else None)


def kernel(**inputs):
    _CACHE.pop("prep_shared", None)
    per_core = [_prep_core_inputs(inputs, c) for c in range(NCORES)]
    res = run_cores(per_core)
    out = np.concatenate([res.results[c]["scores"] for c in range(NCORES)],
                         axis=0)
    return out.astype(np.float32)
